# revision 1
# baseline (speedup 1.0000x reference)
"""Trainium2 Bass kernel for nn_BAR_86045374808446 (sparse_attention).

Math: for each head h (one per NeuronCore, 8 cores):
  s[i,j,d] = ahat_i[d] + bhat_j[d]         (ahat/bhat are d-mean-centered)
  var[i,j] = va[i] + vb[j] + (2/D)<ahat_i, bhat_j>      (matmul!)
  r[i,j]   = 1/sqrt(var + eps)
  out[i,d] = sum_{j<=i} exp(s[i,j,d] * r[i,j])

Factorization (exact to fp32, Taylor order K):
  exp(s*r) = exp(ahat*rbar) * exp(bhat*rbar) * exp(s*w),  w = r - rbar
  exp(s*w) = sum_k (s*w)^k / k! = sum_{p+e=k} w^k * (ahat^p/p!) * (bhat^e/e!)
  => out = sum_p A_p  (*)  sum_e (mask*w^(p+e))^T @ B_e
  with A_p = ahat^p/p! * exp(ahat*rbar)  [i,d],
       B_e = bhat^e/e! * exp(bhat*rbar)  [j,d],
  so the whole T^2*D work is PSUM-accumulated matmuls on the TensorEngine.
"""

import sys

import numpy as np

for _p in ("/opt/trn_rl_repo", "/root/.axon_site/_ro/trn_rl_repo"):
    if _p not in sys.path:
        sys.path.insert(0, _p)

T, D, H, P, NB = 512, 64, 8, 128, 4
K = 11               # taylor order (12 terms)
EPS = 1e-5
CHUNK = (K + 1) * D  # 832 psum cols per i-block

_cached = {}


def _build_nc(use_f32r=True, dump=None):
    import concourse.bass as bass
    import concourse.mybir as mybir
    from concourse import bass_isa
    from concourse.tile import TileContext
    from concourse.masks import make_identity

    f32 = mybir.dt.float32
    f32r = mybir.dt.float32r
    Alu = mybir.AluOpType
    Act = mybir.ActivationFunctionType

    nc = bass.Bass()
    ah_d = nc.declare_dram_parameter("ah", [T, D], f32, isOutput=False)
    bh_d = nc.declare_dram_parameter("bh", [T, D], f32, isOutput=False)
    out_d = nc.declare_dram_parameter("out", [T, D], f32, isOutput=True)
    dbg_d = (nc.declare_dram_parameter("dbg", [P, 4 * T], f32, isOutput=True)
             if dump else None)

    mmdt = f32r if use_f32r else f32

    with TileContext(nc) as tc:
        with (
            tc.tile_pool(name="const", bufs=1) as constp,
            tc.tile_pool(name="work", bufs=1) as work,
            tc.tile_pool(name="wpool", bufs=8) as wpool,
            tc.tile_pool(name="fin", bufs=4) as fin,
            tc.tile_pool(name="psum", bufs=1, space="PSUM") as psum,
        ):
            # ---------------- load ----------------
            Asb = work.tile([P, NB, D], f32, tag="Asb")
            Bsb = work.tile([P, NB, D], f32, tag="Bsb")
            nc.sync.dma_start(out=Asb, in_=ah_d[:].rearrange("(nb p) d -> p nb d", p=P))
            nc.sync.dma_start(out=Bsb, in_=bh_d[:].rearrange("(nb p) d -> p nb d", p=P))

            identity = constp.tile([P, P], f32, tag="ident")
            make_identity(nc, identity)
            eps_col = constp.tile([P, 1], f32, tag="eps")
            nc.vector.memset(eps_col, EPS)
            onesT = constp.tile([P, T], f32, tag="ones")
            nc.gpsimd.memset(onesT, 1.0)
            # warm the ACT Sqrt/Exp tables off the critical path
            warm = constp.tile([P, 1], f32, tag="warm")
            nc.scalar.activation(out=warm, in_=eps_col, func=Act.Sqrt)
            nc.scalar.activation(out=warm, in_=eps_col, func=Act.Exp)

            # ---------------- stats: mean/var per row, center ----------------
            mva = work.tile([P, NB, 2], f32, tag="mva")
            mvb = work.tile([P, NB, 2], f32, tag="mvb")
            A2 = work.tile([P, NB, D], f32, tag="A2")
            Dt = [psum.tile([P, 1024], f32, tag=f"D{ib}", name=f"D{ib}")
                  for ib in range(NB)]
            aT = work.tile([66, NB, P], f32, tag="aT")
            bT = work.tile([66, NB, P], f32, tag="bT")
            for blk in range(NB):
                sa = work.tile([P, 6], f32, tag="bnsA")
                nc.vector.bn_stats(out=sa, in_=Asb[:, blk, :])
                nc.vector.bn_aggr(out=mva[:, blk, :], in_=sa)
                sb = work.tile([P, 6], f32, tag="bnsB")
                nc.vector.bn_stats(out=sb, in_=Bsb[:, blk, :])
                nc.vector.bn_aggr(out=mvb[:, blk, :], in_=sb)
                nc.vector.tensor_scalar(
                    out=Asb[:, blk, :], in0=Asb[:, blk, :],
                    scalar1=mva[:, blk, 0:1], scalar2=None, op0=Alu.subtract)
                nc.vector.tensor_scalar(
                    out=Bsb[:, blk, :], in0=Bsb[:, blk, :],
                    scalar1=mvb[:, blk, 0:1], scalar2=None, op0=Alu.subtract)
                nc.gpsimd.tensor_scalar(out=A2[:, blk, :], in0=Asb[:, blk, :],
                                        scalar1=2.0 / D, scalar2=None,
                                        op0=Alu.mult)
                Ta = work.tile([P, 66], f32, tag="Ta")
                nc.scalar.copy(out=Ta[:, 0:D], in_=A2[:, blk, :])
                nc.gpsimd.memset(Ta[:, D:D + 1], 1.0)
                nc.gpsimd.tensor_copy(out=Ta[:, D + 1:D + 2], in_=mva[:, blk, 1:2])
                Tb = work.tile([P, 66], f32, tag="Tb")
                nc.scalar.copy(out=Tb[:, 0:D], in_=Bsb[:, blk, :])
                nc.gpsimd.tensor_copy(out=Tb[:, D:D + 1], in_=mvb[:, blk, 1:2])
                nc.gpsimd.memset(Tb[:, D + 1:D + 2], 1.0)
                tp = Dt[blk][0:66, 512:512 + P]
                nc.tensor.transpose(tp, Ta, identity)
                nc.vector.tensor_copy(out=aT[:, blk, :], in_=tp)
                tp2 = Dt[blk][0:66, 512 + P:512 + 2 * P]
                nc.tensor.transpose(tp2, Tb, identity)
                nc.vector.tensor_copy(out=bT[:, blk, :], in_=tp2)

            # ---------------- var matmuls -> rT = 1/sqrt(var+eps) -------------
            # varT[j, i] = vb[j] + va[i] + (2/D) sum_d bhatT[d,j] ahatT[d,i]
            rT = work.tile([P, NB, T], f32, tag="rT")
            aT_flat = aT.rearrange("k nb p -> k (nb p)")
            zmx = work.tile([P, NB], f32, tag="zmx")
            zmn = work.tile([P, NB], f32, tag="zmn")
            # m=3 right after m=0 so the global var min/max (-> rbar) is
            # complete two matmuls early and its chain hides under m=1/m=2.
            for m in (0, 3, 1, 2):
                vp = Dt[m][:, 0:T]
                nc.tensor.matmul(vp, bT[:, m, :], aT_flat, start=True, stop=True,
                                 skip_group_check=True)
                nc.vector.tensor_reduce(out=zmx[:, m:m + 1], in_=vp,
                                        axis=mybir.AxisListType.X, op=Alu.max)
                nc.vector.tensor_reduce(out=zmn[:, m:m + 1], in_=vp,
                                        axis=mybir.AxisListType.X, op=Alu.min)
                nc.scalar.activation(out=rT[:, m, :], in_=vp, func=Act.Sqrt,
                                     bias=eps_col, scale=1.0)
                nc.vector.reciprocal(out=rT[:, m, :], in_=rT[:, m, :])
            rT_flat = rT.rearrange("p nb t -> p (nb t)")
            if dump == "r":
                nc.sync.dma_start(out=dbg_d[:], in_=rT_flat)

            # ---------------- rbar, w = r - rbar ------------------------------
            z2 = work.tile([P, 2], f32, tag="z2")
            nc.vector.tensor_reduce(out=z2[:, 0:1], in_=zmx,
                                    axis=mybir.AxisListType.X, op=Alu.max)
            nc.vector.tensor_reduce(out=z2[:, 1:2], in_=zmn,
                                    axis=mybir.AxisListType.X, op=Alu.min)
            nc.vector.tensor_scalar(out=z2[:, 1:2], in0=z2[:, 1:2], scalar1=-1.0,
                                    scalar2=None, op0=Alu.mult)
            # cross-partition: transpose [P,2]->[2,P], reduce free -> [2,1],
            # then rbar = 0.5*max(r) - 0.5*max(-r) broadcast to all partitions
            # via a [2,P] constant matmul (walrus rejects partition_all_reduce).
            ztp = Dt[0][0:2, 768:768 + P]
            nc.tensor.transpose(ztp, z2, identity)
            zrow = work.tile([2, P], f32, tag="zrow")
            nc.vector.tensor_copy(out=zrow, in_=ztp)
            zm = work.tile([2, 1], f32, tag="zm")
            nc.vector.tensor_reduce(out=zm, in_=zrow, axis=mybir.AxisListType.X,
                                    op=Alu.max)
            # zm = [max var, -min var]; restore sign, r = 1/sqrt(v + eps)
            sgn2 = constp.tile([2, 1], f32, tag="sgn2")
            nc.vector.memset(sgn2, 1.0)
            nc.gpsimd.affine_select(out=sgn2, in_=sgn2, compare_op=Alu.is_ge,
                                    fill=-1.0, base=0, channel_multiplier=-1,
                                    pattern=[[0, 1]])
            nc.vector.tensor_scalar(out=zm, in0=zm, scalar1=sgn2, scalar2=None,
                                    op0=Alu.mult)
            nc.scalar.activation(out=zm, in_=zm, func=Act.Sqrt,
                                 bias=eps_col[0:2, :], scale=1.0)
            nc.vector.reciprocal(out=zm, in_=zm)
            half = constp.tile([2, P], f32, tag="half")
            nc.vector.memset(half, 0.5)
            rbp = Dt[1][:, 768:769]
            nc.tensor.matmul(rbp, half, zm, start=True, stop=True,
                             skip_group_check=True)
            rbar = work.tile([P, 1], f32, tag="rbar")
            nc.vector.tensor_copy(out=rbar, in_=rbp)
            # w_m = r_m - rbar (per block, pipelined) and w2_m = w_m^2
            w2 = work.tile([P, NB, T], f32, tag="w2")
            for m in range(NB):
                nc.vector.tensor_scalar(out=rT[:, m, :], in0=rT[:, m, :],
                                        scalar1=rbar, scalar2=None,
                                        op0=Alu.subtract)
                nc.gpsimd.tensor_tensor(out=w2[:, m, P * m:T],
                                        in0=rT[:, m, P * m:T],
                                        in1=rT[:, m, P * m:T], op=Alu.mult)
            if dump == "w":
                nc.sync.dma_start(out=dbg_d[:], in_=rT_flat)
            if dump == "rbar":
                nc.sync.dma_start(out=dbg_d[:, 0:1], in_=rbar)

            # ---------------- A_p, B_e tensors --------------------------------
            # A_all[:, ib, p, :] = ahat^p/p! * exp(ahat*rbar)
            # B_all[:, jb, K-e, :] = bhat^e/e! * exp(bhat*rbar)   (reversed slots)
            A_all = work.tile([P, NB, K + 1, D], f32, tag="A_all")
            B_all = work.tile([P, NB, K + 8, D], mmdt, tag="B_all")
            for nb in range(NB):
                nc.gpsimd.memset(B_all[:, nb, K + 1:K + 8, :].bitcast(f32), 0.0)
            nc.scalar.activation(out=B_all[:, :, K, :], in_=Bsb, func=Act.Exp,
                                 scale=rbar)
            for p_ in range(1, K + 1):
                nc.vector.scalar_tensor_tensor(
                    out=B_all[:, :, K - p_, :], in0=Bsb, scalar=1.0 / p_,
                    in1=B_all[:, :, K - p_ + 1, :], op0=Alu.mult, op1=Alu.mult)
            if dump == "A":
                nc.sync.dma_start(out=dbg_d[:], in_=A_all.rearrange(
                    "p nb k d -> p (nb k d)")[:, 0:4 * T])
            if dump == "B":
                nc.sync.dma_start(out=dbg_d[:], in_=B_all.rearrange(
                    "p nb k d -> p (nb k d)")[:, 0:4 * T].bitcast(f32))

            # ---------------- main loop ----------------------------------------
            def emit_mm(ib, m, k, Wt, last):
                """matmuls for (jblk m, iblock ib, taylor step k).

                start=True resets the whole psum bank, so each bank-region
                gets one full-width start (zero B-slots pad chunks c>k) and
                width-clipped accumulates after that."""
                lhsT = Wt[:, (ib - m) * P:(ib - m) * P + P]
                base = K - k  # slot of chunk c=0
                nseg = (k + 1) * D
                # region A: cols [0, 512) = chunks 0..7
                if m == 0 and k == 0:
                    nc.tensor.matmul(Dt[ib][:, 0:512], lhsT,
                                     B_all[:, m, K:K + 8, :],
                                     start=True, stop=False,
                                     skip_group_check=True)
                else:
                    cA = min(max(nseg, 256), 512)
                    nc.tensor.matmul(Dt[ib][:, 0:cA], lhsT,
                                     B_all[:, m, base:base + cA // D, :],
                                     start=False, stop=last,
                                     skip_group_check=True)
                # region B: cols [512, CHUNK) = chunks 8..12
                if k >= 8:
                    if m == 0 and k == 8:
                        nc.tensor.matmul(Dt[ib][:, 512:CHUNK], lhsT,
                                         B_all[:, m, K:K + (CHUNK - 512) // D, :],
                                         start=True, stop=False,
                                         skip_group_check=True)
                    else:
                        cB = min(max(nseg - 512, 256), CHUNK - 512)
                        nc.tensor.matmul(Dt[ib][:, 512:512 + cB], lhsT,
                                         B_all[:, m, base + 8:base + 8 + cB // D, :],
                                         start=False, stop=last,
                                         skip_group_check=True)

            Wsm = [[None] * (K + 1) for _ in range(NB)]

            def build_w(m, k):
                wm = T - P * m
                Wn = wpool.tile([P, T], mmdt, tag="W", name=f"W{k}_{m}")
                if k == 0:
                    nc.gpsimd.affine_select(
                        out=Wn[:, 0:wm], in_=onesT[:, 0:wm],
                        compare_op=Alu.is_ge, fill=0.0, base=0,
                        channel_multiplier=-1, pattern=[[1, wm]])
                elif k == 1:
                    nc.vector.tensor_tensor(out=Wn[:, 0:wm],
                                            in0=Wsm[m][0][:, 0:wm],
                                            in1=rT[:, m, P * m:T], op=Alu.mult)
                else:
                    # W_k = W_{k-2} * w^2: two chains, split across engines
                    eng = nc.vector if ((k + m) % 2 == 1) else nc.gpsimd
                    eng.tensor_tensor(out=Wn[:, 0:wm],
                                      in0=Wsm[m][k - 2][:, 0:wm],
                                      in1=w2[:, m, P * m:T], op=Alu.mult)
                Wsm[m][k] = Wn

            def emit_final(ib):
                tmp = fin.tile([P, CHUNK], f32, tag="tmp", name=f"tmp{ib}")
                nc.vector.tensor_tensor(out=tmp, in0=A_all[:, ib, :, :],
                                        in1=Dt[ib][:, 0:CHUNK], op=Alu.mult)
                osb = fin.tile([P, D], f32, tag="osb", name=f"osb{ib}")
                nc.vector.tensor_reduce(
                    out=osb, in_=tmp.rearrange("p (s d) -> p d s", s=K + 1),
                    axis=mybir.AxisListType.X, op=Alu.add)
                nc.sync.dma_start(out=out_d[ib * P:(ib + 1) * P, :], in_=osb)

            # m-major: per-jblk k chains; Dt[m] completes at the end of
            # iteration m, so its final is emitted (and runs) right away.
            for m in range(NB):
                for k in range(K + 1):
                    build_w(m, k)
                    for ib in range(m, NB):
                        emit_mm(ib, m, k, Wsm[m][k], last=(m == ib and k == K))
                if m == 0:
                    # A_p tensors (needed only by the finals)
                    nc.scalar.activation(out=A_all[:, :, 0, :], in_=Asb,
                                         func=Act.Exp, scale=rbar)
                    for p_ in range(1, K + 1):
                        nc.vector.scalar_tensor_tensor(
                            out=A_all[:, :, p_, :], in0=Asb, scalar=1.0 / p_,
                            in1=A_all[:, :, p_ - 1, :], op0=Alu.mult,
                            op1=Alu.mult)
                emit_final(m)

            if dump == "D":
                for ib in range(2):
                    dcp = fin.tile([P, CHUNK], f32, tag="dcp", name=f"dcp{ib}")
                    nc.vector.tensor_copy(out=dcp, in_=Dt[ib][:, 0:CHUNK])
                    nc.sync.dma_start(out=dbg_d[:, ib * CHUNK:(ib + 1) * CHUNK],
                                      in_=dcp)


    _split_multi_waits(nc, mybir)
    return nc


def _split_multi_waits(nc, mybir):
    """TRN2 TPB instructions have a single sync-wait slot; walrus cannot
    split >1 wait for several structs. Use the bacc rust pass to split
    them into EventSemaphore instructions."""
    import bass_rust as _bass_rust
    _bass_rust.generate_event_semaphores(nc)
    # walrus rejects wait-only EventSemaphore encodings ("ISA wrong length")
    # and requires update_value == 1. Give each wait-carrier a +1 update of a
    # scratch semaphore nothing ever waits on.
    used = set()
    for f in nc.m.functions:
        for blk in f.blocks:
            for inst in blk.instructions:
                si = getattr(inst, "sync_info", None)
                if si is not None:
                    for w in (si.on_wait or []):
                        used.add(w.id)
                    for u in (si.on_update or []):
                        used.add(u.id)
    scratch = next(s for s in nc._kernel_sem_range if s not in used)
    for f in nc.m.functions:
        for blk in f.blocks:
            for inst in blk.instructions:
                if isinstance(inst, mybir.InstEventSemaphore):
                    si = inst.sync_info
                    if si is not None and si.on_wait and not si.on_update:
                        si.on_update = [_bass_rust.SyncUpdate(
                            sync_type='semaphore', id=scratch,
                            ant_name='wsplit_scratch',
                            update_mode='sem-inc', update_value=1,
                            update_reg=None)]
    # Drop end-of-kernel EVENT_SEMAPHORE_RANGE_CLEAR (opcode 0xb0): this
    # walrus build rejects its encoding ("ISA wrong length"), and the kernel
    # preamble re-clears all kernel semaphores on every run anyway.
    for f in nc.m.functions:
        for blk in f.blocks:
            blk.instructions[:] = [
                inst for inst in blk.instructions
                if not (isinstance(inst, mybir.InstISA)
                        and getattr(inst, "isa_opcode", None) == 0xb0
                        and not (inst.sync_info and
                                 (inst.sync_info.on_wait or
                                  inst.sync_info.on_update)))
            ]


def _get_nc(use_f32r=True, dump=None):
    key = ("nc", use_f32r, dump)
    if key not in _cached:
        _cached[key] = _build_nc(use_f32r, dump)
    return _cached[key]


def kernel(a, b, num_head=8, head_size=64, **kwargs):
    from concourse.bass_utils import run_bass_kernel_spmd

    a = np.asarray(a)
    b = np.asarray(b)
    nc = _get_nc()
    in_maps = []
    for h in range(H):
        in_maps.append({
            "ah": np.ascontiguousarray(a[0, :, h * D:(h + 1) * D], dtype=np.float32),
            "bh": np.ascontiguousarray(b[0, :, h * D:(h + 1) * D], dtype=np.float32),
        })
    res = run_bass_kernel_spmd(nc, in_maps, list(range(H)))
    full = np.concatenate([res.results[h]["out"] for h in range(H)], axis=-1)
    return full[None].astype(np.float32)


if __name__ == "__main__":
    import sys
    sys.path.insert(0, "/opt/trn_rl_repo")
    _build_nc()
    print("build OK")



# revision 15
# speedup vs baseline: 1.4736x; 1.4736x over previous
"""Trainium2 Bass kernel for nn_BAR_86045374808446 (sparse_attention).

Math per head h (one head per NeuronCore, 8 cores):
  s[i,j,d] = ahat_i[d] + bhat_j[d]          (ahat/bhat are d-mean-centered)
  var[i,j] = va[i] + vb[j] + (2/D)<ahat_i, bhat_j>     (one PE matmul per block)
  r[i,j]   = 1/sqrt(var + eps)
  out[i,d] = sum_{j<=i} exp(s[i,j,d] * r[i,j])

Factorization (polynomial P(x) ~ exp(x) on the observed x-range):
  exp(s*r) = exp(ahat*rbar) * exp(bhat*rbar) * exp(s*w),  w = r - rbar
  exp(s*w) ~ P(s*w) = sum_k c_k (s*w)^k
  (s*w)^k  = sum_{p+e=k} k!/(p!e!) ahat^p bhat^e w^k
  => out = sum_p A_p (*) sum_k (M*w^k)^T @ (d_k * B_{k-p}),  d_k = c_k k!
  with A_p = ahat^p/p! * exp(ahat*rbar)  [i,d],
       B_e = bhat^e/e! * exp(bhat*rbar)  [j,d],
  so the T^2*D work is bf16 PSUM-accumulated matmuls on the TensorEngine,
  and the polynomial coefficients ride on pre-scaled bf16 rhs copies (B2).
  rbar = 1/sqrt(mean va + mean vb + eps) -- picked to center the x-range;
  c_k are a Chebyshev fit of exp on that range (error budget 2e-2 rel).
"""

import math
import sys

import numpy as np

for _p in ("/opt/trn_rl_repo", "/root/.axon_site/_ro/trn_rl_repo"):
    if _p not in sys.path:
        sys.path.insert(0, _p)

T, D, H, P, NB = 512, 64, 8, 128, 4
EPS = 1e-5
DEG = 4
COEF = {
    4: [0.99963261, 0.99058825, 0.50079216, 0.18677153, 0.043321831],
    5: [1.00029, 0.99982237, 0.49719599, 0.16689019, 0.045660714,
        0.0085691588],
}

_cached = {}


def _build_nc(deg=DEG, dump=None):
    import concourse.bass as bass
    import concourse.mybir as mybir
    from concourse.tile import TileContext
    from concourse.masks import make_identity

    f32 = mybir.dt.float32
    f32r = mybir.dt.float32r
    bf16 = mybir.dt.bfloat16
    Alu = mybir.AluOpType
    Act = mybir.ActivationFunctionType

    coef = COEF[deg]
    dk = [float(coef[k]) * math.factorial(k) for k in range(deg + 1)]
    CHUNK = (deg + 1) * D

    nc = bass.Bass()
    ah_d = nc.declare_dram_parameter("ah", [T, D], f32, isOutput=False)
    bh_d = nc.declare_dram_parameter("bh", [T, D], f32, isOutput=False)
    out_d = nc.declare_dram_parameter("out", [T, D], f32, isOutput=True)
    dbg_d = (nc.declare_dram_parameter("dbg", [P, 4 * T], f32, isOutput=True)
             if dump else None)

    with TileContext(nc) as tc:
        with (
            tc.tile_pool(name="const", bufs=1) as constp,
            tc.tile_pool(name="work", bufs=1) as work,
            tc.tile_pool(name="wpool", bufs=8) as wpool,
            tc.tile_pool(name="w1pool", bufs=4) as w1pool,
            tc.tile_pool(name="rpool", bufs=2) as rpool,
            tc.tile_pool(name="fin", bufs=4) as fin,
            tc.tile_pool(name="psum", bufs=1, space="PSUM") as psum,
        ):
            # ---------------- load ----------------
            Asb = work.tile([P, NB, D], f32, tag="Asb")
            Bsb = work.tile([P, NB, D], f32, tag="Bsb")
            nc.sync.dma_start(out=Asb, in_=ah_d[:].rearrange("(nb p) d -> p nb d", p=P))
            nc.sync.dma_start(out=Bsb, in_=bh_d[:].rearrange("(nb p) d -> p nb d", p=P))

            # ---------------- constants ----------------
            identity = constp.tile([P, P], f32, tag="ident")
            make_identity(nc, identity)
            eps_col = constp.tile([P, 1], f32, tag="eps")
            nc.vector.memset(eps_col, EPS)
            ones1p = constp.tile([1, P], f32, tag="ones1p")
            nc.vector.memset(ones1p, 1.0)
            ones_bf = constp.tile([P, T], bf16, tag="ones_bf")
            nc.gpsimd.memset(ones_bf, 1.0)
            # causal mask (j<=i within-block pattern; same for every m)
            mask0 = constp.tile([P, T], bf16, tag="mask0")
            nc.gpsimd.affine_select(
                out=mask0, in_=ones_bf, compare_op=Alu.is_ge, fill=0.0,
                base=0, channel_multiplier=-1, pattern=[[1, T]])
            # warm ACT tables off the critical path
            warm = constp.tile([P, 1], f32, tag="warm")
            nc.scalar.activation(out=warm, in_=eps_col, func=Act.Sqrt)
            nc.scalar.activation(out=warm, in_=eps_col, func=Act.Exp)
            nc.scalar.activation(out=warm, in_=eps_col, func=Act.Square)
            nc.scalar.activation(out=warm, in_=eps_col, func=Act.Identity)

            # ---------------- stats + centering ----------------
            mva = work.tile([P, NB, 2], f32, tag="mva")
            mvb = work.tile([P, NB, 2], f32, tag="mvb")
            for blk in range(NB):
                sa = work.tile([P, 6], f32, tag="bnsA", name=f"bnsA{blk}")
                nc.vector.bn_stats(out=sa, in_=Asb[:, blk, :])
                nc.vector.bn_aggr(out=mva[:, blk, :], in_=sa)
            for blk in range(NB):
                sb = work.tile([P, 6], f32, tag="bnsB", name=f"bnsB{blk}")
                nc.vector.bn_stats(out=sb, in_=Bsb[:, blk, :])
                nc.vector.bn_aggr(out=mvb[:, blk, :], in_=sb)
            negmua = work.tile([P, NB, 1], f32, tag="negmua")
            negmub = work.tile([P, NB, 1], f32, tag="negmub")
            nc.vector.tensor_scalar(out=negmua, in0=mva[:, :, 0:1], scalar1=-1.0,
                                    scalar2=None, op0=Alu.mult)
            nc.vector.tensor_scalar(out=negmub, in0=mvb[:, :, 0:1], scalar1=-1.0,
                                    scalar2=None, op0=Alu.mult)
            # centered tensors: ahat standalone; bhat lives inside Tb cols 0:D
            ahat = work.tile([P, NB, D], f32, tag="ahat")
            Ta = work.tile([P, NB, 66], f32, tag="Ta")
            Tb = work.tile([P, NB, 66], f32, tag="Tb")
            bhat = Tb[:, :, 0:D]
            for blk in range(NB):
                nc.scalar.activation(out=ahat[:, blk, :], in_=Asb[:, blk, :],
                                     func=Act.Identity, bias=negmua[:, blk, :])
                nc.scalar.activation(out=Tb[:, blk, 0:D], in_=Bsb[:, blk, :],
                                     func=Act.Identity, bias=negmub[:, blk, :])
            # Ta = [(2/D)*ahat | 1 | va],  Tb = [bhat | vb | 1]
            nc.vector.tensor_scalar(out=Ta[:, :, 0:D], in0=ahat, scalar1=2.0 / D,
                                    scalar2=None, op0=Alu.mult)
            nc.vector.memset(Ta[:, :, D:D + 1], 1.0)
            nc.vector.tensor_copy(out=Ta[:, :, D + 1:D + 2], in_=mva[:, :, 1:2])
            nc.vector.tensor_copy(out=Tb[:, :, D:D + 1], in_=mvb[:, :, 1:2])
            nc.vector.memset(Tb[:, :, D + 1:D + 2], 1.0)

            # ---------------- transposes (PE) ----------------
            scratch = psum.tile([P, 512], f32, tag="scratch")
            aT = work.tile([66, NB, P], f32r, tag="aT")
            bT = work.tile([66, NB, P], f32r, tag="bT")
            for blk in range(NB):
                tp = scratch[:, (blk % 2) * P:(blk % 2) * P + P]
                nc.tensor.transpose(tp[0:66, :], Ta[:, blk, :], identity)
                nc.scalar.activation(out=aT[:, blk, :],
                                     in_=tp[0:66, :], func=Act.Copy)
            for blk in range(NB):
                tp = scratch[:, (2 + blk % 2) * P:(2 + blk % 2) * P + P]
                nc.tensor.transpose(tp[0:66, :], Tb[:, blk, :], identity)
                nc.scalar.activation(out=bT[:, blk, :],
                                     in_=tp[0:66, :], func=Act.Copy)
            aT_flat = aT.rearrange("k nb p -> k (nb p)")

            # ---------------- rbar = 1/sqrt(mean(va)+mean(vb)+eps) ----------
            vs2 = work.tile([P, 2], f32, tag="vs2")
            nc.vector.tensor_reduce(
                out=vs2[:, 0:1],
                in_=mva[:, :, 1:2].rearrange("p nb one -> p (nb one)"),
                axis=mybir.AxisListType.X, op=Alu.add)
            nc.vector.tensor_reduce(
                out=vs2[:, 1:2],
                in_=mvb[:, :, 1:2].rearrange("p nb one -> p (nb one)"),
                axis=mybir.AxisListType.X, op=Alu.add)
            vs1 = work.tile([P, 1], f32, tag="vs1")
            nc.vector.tensor_tensor(out=vs1, in0=vs2[:, 0:1], in1=vs2[:, 1:2],
                                    op=Alu.add)
            tpz = scratch[:, 0:P]
            nc.tensor.transpose(tpz[0:1, :], vs1, identity)
            zrow = work.tile([1, P], f32, tag="zrow")
            nc.scalar.activation(out=zrow, in_=tpz[0:1, :], func=Act.Copy)
            zs = work.tile([1, 1], f32, tag="zs")
            nc.vector.tensor_reduce(out=zs, in_=zrow,
                                    axis=mybir.AxisListType.X, op=Alu.add)
            u1 = work.tile([1, 1], f32, tag="u1")
            nc.scalar.activation(out=u1, in_=zs, func=Act.Sqrt,
                                 bias=eps_col[0:1, :], scale=1.0 / T)
            r1 = work.tile([1, 1], f32, tag="r1")
            nc.vector.reciprocal(out=r1, in_=u1)
            rbp = scratch[:, P:P + 1]
            nc.tensor.matmul(rbp, ones1p, r1, start=True, stop=True,
                             skip_group_check=True)
            rbar = work.tile([P, 1], f32, tag="rbar")
            nc.vector.tensor_copy(out=rbar, in_=rbp)
            if dump == "rbar":
                nc.sync.dma_start(out=dbg_d[:, 0:1], in_=rbar)

            # ---------------- B side: RB slots + scaled bf16 copies ----------
            # RB[:, :, deg-e, :] = B_e = bhat^e/e! * exp(bhat*rbar)
            RB = work.tile([P, NB, deg + 1, D], bf16, tag="RB")
            nc.scalar.activation(out=RB[:, :, deg, :], in_=bhat, func=Act.Exp,
                                 scale=rbar)
            bb2 = work.tile([P, NB, D], bf16, tag="bb2")
            nc.scalar.activation(out=bb2, in_=bhat, func=Act.Square)
            # odd chain on DVE, even chain on Pool
            nc.vector.scalar_tensor_tensor(
                out=RB[:, :, deg - 1, :], in0=bhat, scalar=1.0,
                in1=RB[:, :, deg, :], op0=Alu.mult, op1=Alu.mult)
            nc.vector.scalar_tensor_tensor(
                out=RB[:, :, deg - 2, :], in0=bb2, scalar=0.5,
                in1=RB[:, :, deg, :], op0=Alu.mult, op1=Alu.mult)
            if deg >= 3:
                nc.vector.scalar_tensor_tensor(
                    out=RB[:, :, deg - 3, :], in0=bb2, scalar=1.0 / 6,
                    in1=RB[:, :, deg - 1, :], op0=Alu.mult, op1=Alu.mult)
            if deg >= 4:
                nc.vector.scalar_tensor_tensor(
                    out=RB[:, :, deg - 4, :], in0=bb2, scalar=1.0 / 12,
                    in1=RB[:, :, deg - 2, :], op0=Alu.mult, op1=Alu.mult)
            if deg >= 5:
                nc.vector.scalar_tensor_tensor(
                    out=RB[:, :, deg - 5, :], in0=bb2, scalar=1.0 / 20,
                    in1=RB[:, :, deg - 3, :], op0=Alu.mult, op1=Alu.mult)
            # B2[k] = d_k * [B_k .. B_0]  (bf16 4x tensor_scalar)
            B2 = {}
            for k in range(deg + 1):
                B2[k] = work.tile([P, NB, k + 1, D], bf16, tag=f"B2_{k}",
                                  name=f"B2_{k}")
                nc.vector.tensor_scalar(out=B2[k], in0=RB[:, :, deg - k:, :],
                                        scalar1=dk[k], scalar2=None,
                                        op0=Alu.mult)

            if dump == "B":
                nc.sync.dma_start(
                    out=dbg_d[:, 0:(deg + 1) * NB * D // 2],
                    in_=RB.rearrange("p nb k d -> p (nb k d)").bitcast(f32))

            # ---------------- A side (Pool) ----------------
            A_all = work.tile([P, NB, deg + 1, D], f32, tag="A_all")
            nc.scalar.activation(out=A_all[:, :, 0, :], in_=ahat, func=Act.Exp,
                                 scale=rbar)
            aa2 = work.tile([P, NB, D], f32, tag="aa2")
            nc.scalar.activation(out=aa2, in_=ahat, func=Act.Square)
            nc.vector.scalar_tensor_tensor(
                out=A_all[:, :, 1, :], in0=ahat, scalar=1.0,
                in1=A_all[:, :, 0, :], op0=Alu.mult, op1=Alu.mult)
            nc.vector.scalar_tensor_tensor(
                out=A_all[:, :, 2, :], in0=aa2, scalar=0.5,
                in1=A_all[:, :, 0, :], op0=Alu.mult, op1=Alu.mult)
            if deg >= 3:
                nc.vector.scalar_tensor_tensor(
                    out=A_all[:, :, 3, :], in0=aa2, scalar=1.0 / 6,
                    in1=A_all[:, :, 1, :], op0=Alu.mult, op1=Alu.mult)
            if deg >= 4:
                nc.vector.scalar_tensor_tensor(
                    out=A_all[:, :, 4, :], in0=aa2, scalar=1.0 / 12,
                    in1=A_all[:, :, 2, :], op0=Alu.mult, op1=Alu.mult)
            if deg >= 5:
                nc.vector.scalar_tensor_tensor(
                    out=A_all[:, :, 5, :], in0=aa2, scalar=1.0 / 20,
                    in1=A_all[:, :, 3, :], op0=Alu.mult, op1=Alu.mult)
            if dump == "A":
                nc.sync.dma_start(
                    out=dbg_d[:, 0:(deg + 1) * NB * D],
                    in_=A_all.rearrange("p nb k d -> p (nb k d)"))

            # ---------------- main loop (m-major over j-blocks) --------------
            Dt = [psum.tile([P, 512], f32, tag=f"D{ib}", name=f"D{ib}")
                  for ib in range(NB)]

            rT_all = (work.tile([P, NB, T], f32, tag="rT", name="rT")
                      if dump else None)

            # all variance matmuls + W_1 factors BEFORE the accumulation
            # passes: the var matmul start=True resets its Dt region, so it
            # must never run after main-loop accumulations begin there.
            W1s = []
            for m in range(NB):
                wm = T - P * m
                vp = Dt[m][:, 0:T]
                nc.tensor.matmul(vp, bT[:, m, :], aT_flat, start=True,
                                 stop=True, skip_group_check=True)
                ut = rpool.tile([P, T], f32, tag="ut", name=f"u{m}")
                nc.scalar.activation(out=ut, in_=vp, func=Act.Sqrt,
                                     bias=eps_col, scale=1.0)
                rt = rpool.tile([P, T], f32, tag="rt", name=f"r{m}")
                nc.vector.reciprocal(out=rt, in_=ut)
                if dump:
                    nc.vector.tensor_copy(out=rT_all[:, m, :], in_=rt)
                # W_1 = mask*(r - rbar)
                W1 = w1pool.tile([P, T], bf16, tag="W1", name=f"W1_{m}")
                nc.vector.scalar_tensor_tensor(
                    out=W1[:, 0:wm], in0=rt[:, P * m:T], scalar=rbar,
                    in1=mask0[:, 0:wm], op0=Alu.subtract, op1=Alu.mult)
                W1s.append(W1)

            Wdump = (work.tile([P, 4, T], f32, tag="Wdump", name="Wdump")
                     if dump == "W" else None)
            for m in range(NB):
                wm = T - P * m
                # W chain: W_k = W_a * W_b (a+b=k)
                W = {0: mask0, 1: W1s[m]}
                for k in range(2, deg + 1):
                    a_, b_ = k // 2, k - k // 2
                    W[k] = wpool.tile([P, T], bf16, tag="W", name=f"W{k}_{m}")
                    nc.vector.tensor_tensor(
                        out=W[k][:, 0:wm], in0=W[a_][:, 0:wm],
                        in1=W[b_][:, 0:wm], op=Alu.mult)
                # pass 0 runs k descending: k=deg covers the full CHUNK
                # with start=True, then strictly-nested prefix accumulates.
                # later passes ascend (prefix accumulates into open regions).
                korder = (range(deg, -1, -1) if m == 0
                          else range(deg + 1))
                for k in korder:
                    for ib in range(m, NB):
                        lhsT = W[k][:, (ib - m) * P:(ib - m) * P + P]
                        start = (m == 0 and k == deg)
                        last = (m == ib and k == (deg if m else 0))
                        nc.tensor.matmul(Dt[ib][:, 0:(k + 1) * D], lhsT,
                                         B2[k][:, m, :, :], start=start,
                                         stop=last, skip_group_check=True)
                if dump == "W" and m == 0:
                    for k in range(1, min(deg + 1, 5)):
                        nc.vector.tensor_copy(out=Wdump[:, k - 1, :],
                                              in_=W[k][:, 0:T])
                    nc.sync.dma_start(out=dbg_d[:], in_=Wdump.rearrange(
                        "p f t -> p (f t)"))
                # final for i-block m (its accumulation just completed)
                tmp = fin.tile([P, CHUNK], f32, tag="tmp", name=f"tmp{m}")
                nc.vector.tensor_tensor(out=tmp, in0=A_all[:, m, :, :],
                                        in1=Dt[m][:, 0:CHUNK], op=Alu.mult)
                osb = fin.tile([P, D], f32, tag="osb", name=f"osb{m}")
                nc.vector.tensor_reduce(
                    out=osb, in_=tmp.rearrange("p (s d) -> p d s", s=deg + 1),
                    axis=mybir.AxisListType.X, op=Alu.add)
                nc.sync.dma_start(out=out_d[m * P:(m + 1) * P, :], in_=osb)

            if dump == "r":
                nc.sync.dma_start(out=dbg_d[:], in_=rT_all.rearrange(
                    "p nb t -> p (nb t)"))
            if dump == "D":
                for ib in range(2):
                    dcp = fin.tile([P, CHUNK], f32, tag="dcp", name=f"dcp{ib}")
                    nc.vector.tensor_copy(out=dcp, in_=Dt[ib][:, 0:CHUNK])
                    nc.sync.dma_start(out=dbg_d[:, ib * CHUNK:(ib + 1) * CHUNK],
                                      in_=dcp)

    _split_multi_waits(nc, mybir)
    return nc


def _split_multi_waits(nc, mybir):
    """TRN2 TPB instructions have a single sync-wait slot; walrus cannot
    split >1 wait for several structs. Use the bacc rust pass to split
    them into EventSemaphore instructions."""
    import bass_rust as _bass_rust
    _bass_rust.generate_event_semaphores(nc)
    # walrus rejects wait-only EventSemaphore encodings ("ISA wrong length")
    # and requires update_value == 1. Give each wait-carrier a +1 update of a
    # scratch semaphore nothing ever waits on.
    used = set()
    for f in nc.m.functions:
        for blk in f.blocks:
            for inst in blk.instructions:
                si = getattr(inst, "sync_info", None)
                if si is not None:
                    for w in (si.on_wait or []):
                        used.add(w.id)
                    for u in (si.on_update or []):
                        used.add(u.id)
    scratch = next(s for s in nc._kernel_sem_range if s not in used)
    for f in nc.m.functions:
        for blk in f.blocks:
            for inst in blk.instructions:
                if isinstance(inst, mybir.InstEventSemaphore):
                    si = inst.sync_info
                    if si is not None and si.on_wait and not si.on_update:
                        si.on_update = [_bass_rust.SyncUpdate(
                            sync_type='semaphore', id=scratch,
                            ant_name='wsplit_scratch',
                            update_mode='sem-inc', update_value=1,
                            update_reg=None)]
    # Drop end-of-kernel EVENT_SEMAPHORE_RANGE_CLEAR (opcode 0xb0): this
    # walrus build rejects its encoding ("ISA wrong length"), and the kernel
    # preamble re-clears all kernel semaphores on every run anyway.
    for f in nc.m.functions:
        for blk in f.blocks:
            blk.instructions[:] = [
                inst for inst in blk.instructions
                if not (isinstance(inst, mybir.InstISA)
                        and getattr(inst, "isa_opcode", None) == 0xb0
                        and not (inst.sync_info and
                                 (inst.sync_info.on_wait or
                                  inst.sync_info.on_update)))
            ]


def _get_nc(deg=DEG, dump=None):
    key = ("nc", deg, dump)
    if key not in _cached:
        _cached[key] = _build_nc(deg, dump)
    return _cached[key]


def kernel(a, b, num_head=8, head_size=64, **kwargs):
    from concourse.bass_utils import run_bass_kernel_spmd

    a = np.asarray(a)
    b = np.asarray(b)
    nc = _get_nc()
    in_maps = []
    for h in range(H):
        in_maps.append({
            "ah": np.ascontiguousarray(a[0, :, h * D:(h + 1) * D], dtype=np.float32),
            "bh": np.ascontiguousarray(b[0, :, h * D:(h + 1) * D], dtype=np.float32),
        })
    res = run_bass_kernel_spmd(nc, in_maps, list(range(H)))
    full = np.concatenate([res.results[h]["out"] for h in range(H)], axis=-1)
    return full[None].astype(np.float32)


if __name__ == "__main__":
    sys.path.insert(0, "/opt/trn_rl_repo")
    _build_nc()
    print("build OK")


# revision 16
# speedup vs baseline: 1.7244x; 1.1702x over previous
"""Trainium2 Bass kernel for nn_BAR_86045374808446 (sparse_attention).

Math per head h (one head per NeuronCore, 8 cores):
  s[i,j,d] = ahat_i[d] + bhat_j[d]          (ahat/bhat are d-mean-centered)
  var[i,j] = va[i] + vb[j] + (2/D)<ahat_i, bhat_j>     (one PE matmul per block)
  r[i,j]   = 1/sqrt(var + eps)
  out[i,d] = sum_{j<=i} exp(s[i,j,d] * r[i,j])

Factorization (polynomial P(x) ~ exp(x) on the observed x-range):
  exp(s*r) = exp(ahat*rbar) * exp(bhat*rbar) * exp(s*w),  w = r - rbar
  exp(s*w) ~ P(s*w) = sum_k c_k (s*w)^k
  (s*w)^k  = sum_{p+e=k} k!/(p!e!) ahat^p bhat^e w^k
  => out = sum_p A_p (*) sum_k (M*w^k)^T @ (d_k * B_{k-p}),  d_k = c_k k!
  with A_p = ahat^p/p! * exp(ahat*rbar)  [i,d],
       B_e = bhat^e/e! * exp(bhat*rbar)  [j,d],
  so the T^2*D work is bf16 PSUM-accumulated matmuls on the TensorEngine,
  and the polynomial coefficients ride on pre-scaled bf16 rhs copies (B2).
  rbar = 1/sqrt(mean va + mean vb + eps) -- picked to center the x-range;
  c_k are a Chebyshev fit of exp on that range (error budget 2e-2 rel).
"""

import math
import sys

import numpy as np

for _p in ("/opt/trn_rl_repo", "/root/.axon_site/_ro/trn_rl_repo"):
    if _p not in sys.path:
        sys.path.insert(0, _p)

T, D, H, P, NB = 512, 64, 8, 128, 4
EPS = 1e-5
DEG = 4
COEF = {
    4: [0.99963261, 0.99058825, 0.50079216, 0.18677153, 0.043321831],
    5: [1.00029, 0.99982237, 0.49719599, 0.16689019, 0.045660714,
        0.0085691588],
}

_cached = {}


def _build_nc(deg=DEG, dump=None):
    import concourse.bass as bass
    import concourse.mybir as mybir
    from concourse.tile import TileContext
    from concourse.masks import make_identity

    f32 = mybir.dt.float32
    f32r = mybir.dt.float32r
    bf16 = mybir.dt.bfloat16
    Alu = mybir.AluOpType
    Act = mybir.ActivationFunctionType

    coef = COEF[deg]
    dk = [float(coef[k]) * math.factorial(k) for k in range(deg + 1)]
    CHUNK = (deg + 1) * D

    nc = bass.Bass()
    ah_d = nc.declare_dram_parameter("ah", [T, D], f32, isOutput=False)
    bh_d = nc.declare_dram_parameter("bh", [T, D], f32, isOutput=False)
    out_d = nc.declare_dram_parameter("out", [T, D], f32, isOutput=True)
    dbg_d = (nc.declare_dram_parameter("dbg", [P, 4 * T], f32, isOutput=True)
             if dump else None)

    with TileContext(nc) as tc:
        with (
            tc.tile_pool(name="const", bufs=1) as constp,
            tc.tile_pool(name="work", bufs=1) as work,
            tc.tile_pool(name="wpool", bufs=8) as wpool,
            tc.tile_pool(name="w1pool", bufs=4) as w1pool,
            tc.tile_pool(name="rpool", bufs=2) as rpool,
            tc.tile_pool(name="fin", bufs=4) as fin,
            tc.tile_pool(name="psum", bufs=1, space="PSUM") as psum,
            tc.tile_pool(name="psumR", bufs=1, space="PSUM") as psumR,
        ):
            # ------- loads: two independent DMA queues (SP + Pool) ----------
            Asb = work.tile([P, NB, D], f32, tag="Asb")
            Bsb = work.tile([P, NB, D], f32, tag="Bsb")
            nc.sync.dma_start(out=Asb,
                              in_=ah_d[:].rearrange("(nb p) d -> p nb d", p=P))
            nc.gpsimd.dma_start(out=Bsb,
                                in_=bh_d[:].rearrange("(nb p) d -> p nb d", p=P))

            # ------- constants ----------------------------------------------
            identity = constp.tile([P, P], f32, tag="ident")
            make_identity(nc, identity)
            eps_col = constp.tile([P, 1], f32, tag="eps")
            nc.vector.memset(eps_col, EPS)
            ones1p = constp.tile([1, P], f32, tag="ones1p")
            nc.vector.memset(ones1p, 1.0)
            ones_bf = constp.tile([P, T], bf16, tag="ones_bf")
            nc.gpsimd.memset(ones_bf, 1.0)
            # causal mask (j<=i within-block pattern; same for every m)
            mask0 = constp.tile([P, T], bf16, tag="mask0")
            nc.gpsimd.affine_select(
                out=mask0, in_=ones_bf, compare_op=Alu.is_ge, fill=0.0,
                base=0, channel_multiplier=-1, pattern=[[1, T]])
            # warm ACT tables off the critical path
            warm = constp.tile([P, 1], f32, tag="warm")
            nc.scalar.activation(out=warm, in_=eps_col, func=Act.Sqrt)
            nc.scalar.activation(out=warm, in_=eps_col, func=Act.Exp)
            nc.scalar.activation(out=warm, in_=eps_col, func=Act.Square)
            nc.scalar.activation(out=warm, in_=eps_col, func=Act.Identity)

            # PSUM scratch bank for transposes; rbar broadcast in its own bank
            scratch = psum.tile([P, 512], f32, tag="scratch")
            rbp = psumR.tile([P, 8], f32, tag="rbp")

            # PE pstate warm-up: back-to-back identity transposes keep the
            # tensor engine ramping while the DMA/stats preamble runs.
            for i in range(10):
                tp = scratch[:, 256 + (i % 2) * P:256 + (i % 2) * P + P]
                nc.tensor.transpose(tp, identity, identity)

            # ------- stats + centering (a first, then b) --------------------
            mva = work.tile([P, NB, 2], f32, tag="mva")
            mvb = work.tile([P, NB, 2], f32, tag="mvb")
            negmua = work.tile([P, NB, 1], f32, tag="negmua")
            negmub = work.tile([P, NB, 1], f32, tag="negmub")
            ahat = work.tile([P, NB, D], f32, tag="ahat")
            Ta = work.tile([P, NB, 66], f32, tag="Ta")
            Tb = work.tile([P, NB, 66], f32, tag="Tb")
            bhat = Tb[:, :, 0:D]

            for blk in range(NB):
                sa = work.tile([P, 6], f32, tag="bnsA", name=f"bnsA{blk}")
                nc.vector.bn_stats(out=sa, in_=Asb[:, blk, :])
                nc.vector.bn_aggr(out=mva[:, blk, :], in_=sa)
            nc.vector.tensor_scalar(out=negmua, in0=mva[:, :, 0:1], scalar1=-1.0,
                                    scalar2=None, op0=Alu.mult)
            for blk in range(NB):
                nc.scalar.activation(out=ahat[:, blk, :], in_=Asb[:, blk, :],
                                     func=Act.Identity, bias=negmua[:, blk, :])
            # Ta = [(2/D)*ahat | 1 | va]
            nc.vector.tensor_scalar(out=Ta[:, :, 0:D], in0=ahat, scalar1=2.0 / D,
                                    scalar2=None, op0=Alu.mult)
            nc.vector.memset(Ta[:, :, D:D + 1], 1.0)
            nc.vector.tensor_copy(out=Ta[:, :, D + 1:D + 2], in_=mva[:, :, 1:2])

            for blk in range(NB):
                sb = work.tile([P, 6], f32, tag="bnsB", name=f"bnsB{blk}")
                nc.vector.bn_stats(out=sb, in_=Bsb[:, blk, :])
                nc.vector.bn_aggr(out=mvb[:, blk, :], in_=sb)
            nc.vector.tensor_scalar(out=negmub, in0=mvb[:, :, 0:1], scalar1=-1.0,
                                    scalar2=None, op0=Alu.mult)
            for blk in range(NB):
                nc.scalar.activation(out=Tb[:, blk, 0:D], in_=Bsb[:, blk, :],
                                     func=Act.Identity, bias=negmub[:, blk, :])
            # Tb = [bhat | vb | 1]
            nc.vector.tensor_copy(out=Tb[:, :, D:D + 1], in_=mvb[:, :, 1:2])
            nc.vector.memset(Tb[:, :, D + 1:D + 2], 1.0)

            # ------- rbar = 1/sqrt(mean(va)+mean(vb)+eps) -------------------
            vs2 = work.tile([P, 2], f32, tag="vs2")
            nc.vector.tensor_reduce(
                out=vs2[:, 0:1],
                in_=mva[:, :, 1:2].rearrange("p nb one -> p (nb one)"),
                axis=mybir.AxisListType.X, op=Alu.add)
            nc.vector.tensor_reduce(
                out=vs2[:, 1:2],
                in_=mvb[:, :, 1:2].rearrange("p nb one -> p (nb one)"),
                axis=mybir.AxisListType.X, op=Alu.add)
            vs1 = work.tile([P, 1], f32, tag="vs1")
            nc.vector.tensor_tensor(out=vs1, in0=vs2[:, 0:1], in1=vs2[:, 1:2],
                                    op=Alu.add)
            tpz = scratch[:, 0:P]
            nc.tensor.transpose(tpz[0:1, :], vs1, identity)
            zrow = work.tile([1, P], f32, tag="zrow")
            nc.scalar.activation(out=zrow, in_=tpz[0:1, :], func=Act.Copy)
            zs = work.tile([1, 1], f32, tag="zs")
            nc.vector.tensor_reduce(out=zs, in_=zrow,
                                    axis=mybir.AxisListType.X, op=Alu.add)
            u1 = work.tile([1, 1], f32, tag="u1")
            nc.scalar.activation(out=u1, in_=zs, func=Act.Sqrt,
                                 bias=eps_col[0:1, :], scale=1.0 / T)
            r1 = work.tile([1, 1], f32, tag="r1")
            nc.vector.reciprocal(out=r1, in_=u1)
            nc.tensor.matmul(rbp[:, 0:1], ones1p, r1, start=True, stop=True,
                             skip_group_check=True)
            rbar = work.tile([P, 1], f32, tag="rbar")
            nc.vector.tensor_copy(out=rbar, in_=rbp[:, 0:1])
            if dump == "rbar":
                nc.sync.dma_start(out=dbg_d[:, 0:1], in_=rbar)

            # ------- transposes (PE): a-side, then b-side -------------------
            aT = work.tile([66, NB, P], f32r, tag="aT")
            bT = work.tile([66, NB, P], f32r, tag="bT")
            for blk in range(NB):
                tp = scratch[:, (blk % 2) * P:(blk % 2) * P + P]
                nc.tensor.transpose(tp[0:66, :], Ta[:, blk, :], identity)
                nc.scalar.activation(out=aT[:, blk, :],
                                     in_=tp[0:66, :], func=Act.Copy)
            for blk in range(NB):
                tp = scratch[:, (2 + blk % 2) * P:(2 + blk % 2) * P + P]
                nc.tensor.transpose(tp[0:66, :], Tb[:, blk, :], identity)
                nc.scalar.activation(out=bT[:, blk, :],
                                     in_=tp[0:66, :], func=Act.Copy)
            aT_flat = aT.rearrange("k nb p -> k (nb p)")

            # ------- B side: RB slots + scaled bf16 copies (desc k) ---------
            # RB[:, :, deg-e, :] = B_e = bhat^e/e! * exp(bhat*rbar)
            RB = work.tile([P, NB, deg + 1, D], bf16, tag="RB")
            nc.scalar.activation(out=RB[:, :, deg, :], in_=bhat, func=Act.Exp,
                                 scale=rbar)
            bb2 = work.tile([P, NB, D], bf16, tag="bb2")
            nc.scalar.activation(out=bb2, in_=bhat, func=Act.Square)
            nc.vector.scalar_tensor_tensor(
                out=RB[:, :, deg - 1, :], in0=bhat, scalar=1.0,
                in1=RB[:, :, deg, :], op0=Alu.mult, op1=Alu.mult)
            nc.vector.scalar_tensor_tensor(
                out=RB[:, :, deg - 2, :], in0=bb2, scalar=0.5,
                in1=RB[:, :, deg, :], op0=Alu.mult, op1=Alu.mult)
            if deg >= 3:
                nc.vector.scalar_tensor_tensor(
                    out=RB[:, :, deg - 3, :], in0=bb2, scalar=1.0 / 6,
                    in1=RB[:, :, deg - 1, :], op0=Alu.mult, op1=Alu.mult)
            if deg >= 4:
                nc.vector.scalar_tensor_tensor(
                    out=RB[:, :, deg - 4, :], in0=bb2, scalar=1.0 / 12,
                    in1=RB[:, :, deg - 2, :], op0=Alu.mult, op1=Alu.mult)
            if deg >= 5:
                nc.vector.scalar_tensor_tensor(
                    out=RB[:, :, deg - 5, :], in0=bb2, scalar=1.0 / 20,
                    in1=RB[:, :, deg - 3, :], op0=Alu.mult, op1=Alu.mult)
            # B2[k] = d_k * [B_k .. B_0]; built descending so the k=deg
            # full-width start matmuls unblock first.
            B2 = {}
            for k in range(deg, -1, -1):
                B2[k] = work.tile([P, NB, k + 1, D], bf16, tag=f"B2_{k}",
                                  name=f"B2_{k}")
                nc.vector.tensor_scalar(out=B2[k], in0=RB[:, :, deg - k:, :],
                                        scalar1=dk[k], scalar2=None,
                                        op0=Alu.mult)
            if dump == "B":
                nc.sync.dma_start(
                    out=dbg_d[:, 0:(deg + 1) * NB * D // 2],
                    in_=RB.rearrange("p nb k d -> p (nb k d)").bitcast(f32))

            # ------- var matmuls -> r_m -> W1_m ------------------------------
            Dt = [psum.tile([P, 512], f32, tag=f"D{ib}", name=f"D{ib}")
                  for ib in range(NB)]
            rT_all = (work.tile([P, NB, T], f32, tag="rT", name="rT")
                      if dump else None)
            W1s = []
            for m in range(NB):
                wm = T - P * m
                vp = Dt[m][:, 0:T]
                nc.tensor.matmul(vp, bT[:, m, :], aT_flat, start=True,
                                 stop=True, skip_group_check=True)
                ut = rpool.tile([P, T], f32, tag="ut", name=f"u{m}")
                nc.scalar.activation(out=ut, in_=vp, func=Act.Sqrt,
                                     bias=eps_col, scale=1.0)
                rt = rpool.tile([P, T], f32, tag="rt", name=f"r{m}")
                nc.vector.reciprocal(out=rt, in_=ut)
                if dump:
                    nc.vector.tensor_copy(out=rT_all[:, m, :], in_=rt)
                # W_1 = mask*(r - rbar)
                W1 = w1pool.tile([P, T], bf16, tag="W1", name=f"W1_{m}")
                nc.vector.scalar_tensor_tensor(
                    out=W1[:, 0:wm], in0=rt[:, P * m:T], scalar=rbar,
                    in1=mask0[:, 0:wm], op0=Alu.subtract, op1=Alu.mult)
                W1s.append(W1)

            # ------- A side (consumed by the finals; can lag) ---------------
            A_all = work.tile([P, NB, deg + 1, D], f32, tag="A_all")
            nc.scalar.activation(out=A_all[:, :, 0, :], in_=ahat, func=Act.Exp,
                                 scale=rbar)
            aa2 = work.tile([P, NB, D], f32, tag="aa2")
            nc.scalar.activation(out=aa2, in_=ahat, func=Act.Square)
            nc.vector.scalar_tensor_tensor(
                out=A_all[:, :, 1, :], in0=ahat, scalar=1.0,
                in1=A_all[:, :, 0, :], op0=Alu.mult, op1=Alu.mult)
            nc.vector.scalar_tensor_tensor(
                out=A_all[:, :, 2, :], in0=aa2, scalar=0.5,
                in1=A_all[:, :, 0, :], op0=Alu.mult, op1=Alu.mult)
            if deg >= 3:
                nc.vector.scalar_tensor_tensor(
                    out=A_all[:, :, 3, :], in0=aa2, scalar=1.0 / 6,
                    in1=A_all[:, :, 1, :], op0=Alu.mult, op1=Alu.mult)
            if deg >= 4:
                nc.vector.scalar_tensor_tensor(
                    out=A_all[:, :, 4, :], in0=aa2, scalar=1.0 / 12,
                    in1=A_all[:, :, 2, :], op0=Alu.mult, op1=Alu.mult)
            if deg >= 5:
                nc.vector.scalar_tensor_tensor(
                    out=A_all[:, :, 5, :], in0=aa2, scalar=1.0 / 20,
                    in1=A_all[:, :, 3, :], op0=Alu.mult, op1=Alu.mult)
            if dump == "A":
                nc.sync.dma_start(
                    out=dbg_d[:, 0:(deg + 1) * NB * D],
                    in_=A_all.rearrange("p nb k d -> p (nb k d)"))

            # ------- main accumulation passes (m-major) ---------------------
            Wdump = (work.tile([P, 4, T], f32, tag="Wdump", name="Wdump")
                     if dump == "W" else None)

            def emit_final(m):
                tmp = fin.tile([P, CHUNK], f32, tag="tmp", name=f"tmp{m}")
                nc.vector.tensor_tensor(out=tmp, in0=A_all[:, m, :, :],
                                        in1=Dt[m][:, 0:CHUNK], op=Alu.mult)
                osb = fin.tile([P, D], f32, tag="osb", name=f"osb{m}")
                if m < NB - 1:
                    # off the critical path: binary add tree on idle Pool
                    t3 = fin.tile([P, 2, D], f32, tag="t3", name=f"t3_{m}")
                    nc.gpsimd.tensor_tensor(
                        out=t3, in0=tmp.rearrange("p (s d) -> p s d", s=deg + 1)[:, 0:2, :],
                        in1=tmp.rearrange("p (s d) -> p s d", s=deg + 1)[:, 2:4, :],
                        op=Alu.add)
                    nc.gpsimd.tensor_tensor(out=t3[:, 0, :], in0=t3[:, 0, :],
                                            in1=t3[:, 1, :], op=Alu.add)
                    nc.gpsimd.tensor_tensor(out=osb, in0=t3[:, 0, :],
                                            in1=tmp[:, deg * D:(deg + 1) * D],
                                            op=Alu.add)
                else:
                    nc.vector.tensor_reduce(
                        out=osb,
                        in_=tmp.rearrange("p (s d) -> p d s", s=deg + 1),
                        axis=mybir.AxisListType.X, op=Alu.add)
                eng = nc.sync if m % 2 == 0 else nc.gpsimd
                eng.dma_start(out=out_d[m * P:(m + 1) * P, :], in_=osb)

            for m in range(NB):
                wm = T - P * m
                # W chain in build order 2,4,3[,5]; consumed descending
                W = {0: mask0, 1: W1s[m]}
                build = [(2, 1, 1), (4, 2, 2), (3, 1, 2)]
                if deg == 5:
                    build.append((5, 2, 3))
                for k, a_, b_ in build[:deg - 1]:
                    W[k] = wpool.tile([P, T], bf16, tag="W", name=f"W{k}_{m}")
                    nc.vector.tensor_tensor(
                        out=W[k][:, 0:wm], in0=W[a_][:, 0:wm],
                        in1=W[b_][:, 0:wm], op=Alu.mult)
                # pass 0 runs k descending: k=deg covers the full CHUNK with
                # start=True, then strictly-nested prefix accumulates.
                korder = (range(deg, -1, -1) if m == 0 else range(deg + 1))
                for k in korder:
                    for ib in range(m, NB):
                        lhsT = W[k][:, (ib - m) * P:(ib - m) * P + P]
                        start = (m == 0 and k == deg)
                        last = (m == ib and k == (deg if m else 0))
                        nc.tensor.matmul(Dt[ib][:, 0:(k + 1) * D], lhsT,
                                         B2[k][:, m, :, :], start=start,
                                         stop=last, skip_group_check=True)
                if dump == "W" and m == 0:
                    for k in range(1, min(deg + 1, 5)):
                        nc.vector.tensor_copy(out=Wdump[:, k - 1, :],
                                              in_=W[k][:, 0:T])
                    nc.sync.dma_start(out=dbg_d[:], in_=Wdump.rearrange(
                        "p f t -> p (f t)"))
                emit_final(m)

            if dump == "r":
                nc.sync.dma_start(out=dbg_d[:], in_=rT_all.rearrange(
                    "p nb t -> p (nb t)"))
            if dump == "D":
                for ib in range(2):
                    dcp = fin.tile([P, CHUNK], f32, tag="dcp", name=f"dcp{ib}")
                    nc.vector.tensor_copy(out=dcp, in_=Dt[ib][:, 0:CHUNK])
                    nc.sync.dma_start(out=dbg_d[:, ib * CHUNK:(ib + 1) * CHUNK],
                                      in_=dcp)

    _split_multi_waits(nc, mybir)
    return nc


def _split_multi_waits(nc, mybir):
    """TRN2 TPB instructions have a single sync-wait slot; walrus cannot
    split >1 wait for several structs. Use the bacc rust pass to split
    them into EventSemaphore instructions."""
    import bass_rust as _bass_rust
    _bass_rust.generate_event_semaphores(nc)
    # walrus rejects wait-only EventSemaphore encodings ("ISA wrong length")
    # and requires update_value == 1. Give each wait-carrier a +1 update of a
    # scratch semaphore nothing ever waits on.
    used = set()
    for f in nc.m.functions:
        for blk in f.blocks:
            for inst in blk.instructions:
                si = getattr(inst, "sync_info", None)
                if si is not None:
                    for w in (si.on_wait or []):
                        used.add(w.id)
                    for u in (si.on_update or []):
                        used.add(u.id)
    scratch = next(s for s in nc._kernel_sem_range if s not in used)
    for f in nc.m.functions:
        for blk in f.blocks:
            for inst in blk.instructions:
                if isinstance(inst, mybir.InstEventSemaphore):
                    si = inst.sync_info
                    if si is not None and si.on_wait and not si.on_update:
                        si.on_update = [_bass_rust.SyncUpdate(
                            sync_type='semaphore', id=scratch,
                            ant_name='wsplit_scratch',
                            update_mode='sem-inc', update_value=1,
                            update_reg=None)]
    # Drop end-of-kernel EVENT_SEMAPHORE_RANGE_CLEAR (opcode 0xb0): this
    # walrus build rejects its encoding ("ISA wrong length"), and the kernel
    # preamble re-clears all kernel semaphores on every run anyway.
    for f in nc.m.functions:
        for blk in f.blocks:
            blk.instructions[:] = [
                inst for inst in blk.instructions
                if not (isinstance(inst, mybir.InstISA)
                        and getattr(inst, "isa_opcode", None) == 0xb0
                        and not (inst.sync_info and
                                 (inst.sync_info.on_wait or
                                  inst.sync_info.on_update)))
            ]


def _get_nc(deg=DEG, dump=None):
    key = ("nc", deg, dump)
    if key not in _cached:
        _cached[key] = _build_nc(deg, dump)
    return _cached[key]


def kernel(a, b, num_head=8, head_size=64, **kwargs):
    from concourse.bass_utils import run_bass_kernel_spmd

    a = np.asarray(a)
    b = np.asarray(b)
    nc = _get_nc()
    in_maps = []
    for h in range(H):
        in_maps.append({
            "ah": np.ascontiguousarray(a[0, :, h * D:(h + 1) * D], dtype=np.float32),
            "bh": np.ascontiguousarray(b[0, :, h * D:(h + 1) * D], dtype=np.float32),
        })
    res = run_bass_kernel_spmd(nc, in_maps, list(range(H)))
    full = np.concatenate([res.results[h]["out"] for h in range(H)], axis=-1)
    return full[None].astype(np.float32)


if __name__ == "__main__":
    sys.path.insert(0, "/opt/trn_rl_repo")
    _build_nc()
    print("build OK")


# revision 17
# speedup vs baseline: 1.9474x; 1.1293x over previous
"""Trainium2 Bass kernel for nn_BAR_86045374808446 (sparse_attention).

Math per head h (one head per NeuronCore, 8 cores):
  s[i,j,d] = ahat_i[d] + bhat_j[d]          (ahat/bhat are d-mean-centered)
  var[i,j] = va[i] + vb[j] + (2/D)<ahat_i, bhat_j>     (one PE matmul per block)
  r[i,j]   = 1/sqrt(var + eps)
  out[i,d] = sum_{j<=i} exp(s[i,j,d] * r[i,j])

Factorization (polynomial P(x) ~ exp(x) on the observed x-range):
  exp(s*r) = exp(ahat*rbar) * exp(bhat*rbar) * exp(s*w),  w = r - rbar
  exp(s*w) ~ P(s*w) = sum_k c_k (s*w)^k
  (s*w)^k  = sum_{p+e=k} k!/(p!e!) ahat^p bhat^e w^k
  => out = sum_p A_p (*) sum_k (M*w^k)^T @ (d_k * B_{k-p}),  d_k = c_k k!
  with A_p = ahat^p/p! * exp(ahat*rbar)  [i,d],
       B_e = bhat^e/e! * exp(bhat*rbar)  [j,d],
  so the T^2*D work is bf16 PSUM-accumulated matmuls on the TensorEngine,
  and the polynomial coefficients ride on pre-scaled bf16 rhs copies (B2).
  rbar = 1/sqrt(mean va + mean vb + eps) -- picked to center the x-range;
  c_k are a Chebyshev fit of exp on that range (error budget 2e-2 rel).
"""

import math
import sys

import numpy as np

for _p in ("/opt/trn_rl_repo", "/root/.axon_site/_ro/trn_rl_repo"):
    if _p not in sys.path:
        sys.path.insert(0, _p)

T, D, H, P, NB = 512, 64, 8, 128, 4
EPS = 1e-5
DEG = 4
COEF = {
    4: [0.99963261, 0.99058825, 0.50079216, 0.18677153, 0.043321831],
    5: [1.00029, 0.99982237, 0.49719599, 0.16689019, 0.045660714,
        0.0085691588],
}

_cached = {}


def _build_nc(deg=DEG, dump=None):
    import concourse.bass as bass
    import concourse.mybir as mybir
    from concourse.tile import TileContext
    from concourse.masks import make_identity

    f32 = mybir.dt.float32
    f32r = mybir.dt.float32r
    bf16 = mybir.dt.bfloat16
    Alu = mybir.AluOpType
    Act = mybir.ActivationFunctionType

    coef = COEF[deg]
    dk = [float(coef[k]) * math.factorial(k) for k in range(deg + 1)]
    CHUNK = (deg + 1) * D

    nc = bass.Bass()
    ah_d = nc.declare_dram_parameter("ah", [T, D], f32, isOutput=False)
    bh_d = nc.declare_dram_parameter("bh", [T, D], f32, isOutput=False)
    out_d = nc.declare_dram_parameter("out", [T, D], f32, isOutput=True)
    dbg_d = (nc.declare_dram_parameter("dbg", [P, 4 * T], f32, isOutput=True)
             if dump else None)

    with TileContext(nc) as tc:
        with (
            tc.tile_pool(name="const", bufs=1) as constp,
            tc.tile_pool(name="work", bufs=1) as work,
            tc.tile_pool(name="wpool", bufs=8) as wpool,
            tc.tile_pool(name="w1pool", bufs=4) as w1pool,
            tc.tile_pool(name="rpool", bufs=2) as rpool,
            tc.tile_pool(name="fin", bufs=4) as fin,
            tc.tile_pool(name="psum", bufs=1, space="PSUM") as psum,
            tc.tile_pool(name="psumR", bufs=1, space="PSUM") as psumR,
        ):
            # ------- loads: two independent DMA queues (SP + Pool) ----------
            Asb = work.tile([P, NB, D], f32, tag="Asb")
            Bsb = work.tile([P, NB, D], f32, tag="Bsb")
            nc.sync.dma_start(out=Asb,
                              in_=ah_d[:].rearrange("(nb p) d -> p nb d", p=P))
            nc.scalar.dma_start(out=Bsb,
                                in_=bh_d[:].rearrange("(nb p) d -> p nb d", p=P))

            # ------- constants ----------------------------------------------
            identity = constp.tile([P, P], f32, tag="ident")
            make_identity(nc, identity)
            eps_col = constp.tile([P, 1], f32, tag="eps")
            nc.vector.memset(eps_col, EPS)
            ones1p = constp.tile([1, P], f32, tag="ones1p")
            nc.vector.memset(ones1p, 1.0)
            ones_bf = constp.tile([P, T], bf16, tag="ones_bf")
            nc.gpsimd.memset(ones_bf, 1.0)
            # causal mask (j<=i within-block pattern; same for every m)
            mask0 = constp.tile([P, T], bf16, tag="mask0")
            nc.gpsimd.affine_select(
                out=mask0, in_=ones_bf, compare_op=Alu.is_ge, fill=0.0,
                base=0, channel_multiplier=-1, pattern=[[1, T]])
            # warm ACT tables off the critical path
            warm = constp.tile([P, 1], f32, tag="warm")
            nc.scalar.activation(out=warm, in_=eps_col, func=Act.Sqrt)
            nc.scalar.activation(out=warm, in_=eps_col, func=Act.Exp)
            nc.scalar.activation(out=warm, in_=eps_col, func=Act.Square)
            nc.scalar.activation(out=warm, in_=eps_col, func=Act.Identity)

            # PSUM scratch bank for transposes; rbar broadcast in its own bank
            scratch = psum.tile([P, 512], f32, tag="scratch")
            rbp = psumR.tile([P, 8], f32, tag="rbp")

            # PE pstate warm-up: back-to-back identity transposes keep the
            # tensor engine ramping while the DMA/stats preamble runs.
            for i in range(10):
                tp = scratch[:, 256 + (i % 2) * P:256 + (i % 2) * P + P]
                nc.tensor.transpose(tp, identity, identity)

            # ------- stats + centering (a first, then b) --------------------
            mva = work.tile([P, NB, 2], f32, tag="mva")
            mvb = work.tile([P, NB, 2], f32, tag="mvb")
            negmua = work.tile([P, NB, 1], f32, tag="negmua")
            negmub = work.tile([P, NB, 1], f32, tag="negmub")
            ahat = work.tile([P, NB, D], f32, tag="ahat")
            Ta = work.tile([P, NB, 66], f32, tag="Ta")
            Tb = work.tile([P, NB, 66], f32, tag="Tb")
            bhat = Tb[:, :, 0:D]

            for blk in range(NB):
                sa = work.tile([P, 6], f32, tag="bnsA", name=f"bnsA{blk}")
                nc.vector.bn_stats(out=sa, in_=Asb[:, blk, :])
                nc.vector.bn_aggr(out=mva[:, blk, :], in_=sa)
            nc.vector.tensor_scalar(out=negmua, in0=mva[:, :, 0:1], scalar1=-1.0,
                                    scalar2=None, op0=Alu.mult)
            for blk in range(NB):
                nc.scalar.activation(out=ahat[:, blk, :], in_=Asb[:, blk, :],
                                     func=Act.Identity, bias=negmua[:, blk, :])
            # Ta = [(2/D)*ahat | 1 | va]
            nc.vector.tensor_scalar(out=Ta[:, :, 0:D], in0=ahat, scalar1=2.0 / D,
                                    scalar2=None, op0=Alu.mult)
            nc.vector.memset(Ta[:, :, D:D + 1], 1.0)
            nc.vector.tensor_copy(out=Ta[:, :, D + 1:D + 2], in_=mva[:, :, 1:2])

            for blk in range(NB):
                sb = work.tile([P, 6], f32, tag="bnsB", name=f"bnsB{blk}")
                nc.vector.bn_stats(out=sb, in_=Bsb[:, blk, :])
                nc.vector.bn_aggr(out=mvb[:, blk, :], in_=sb)
            nc.vector.tensor_scalar(out=negmub, in0=mvb[:, :, 0:1], scalar1=-1.0,
                                    scalar2=None, op0=Alu.mult)
            for blk in range(NB):
                nc.scalar.activation(out=Tb[:, blk, 0:D], in_=Bsb[:, blk, :],
                                     func=Act.Identity, bias=negmub[:, blk, :])
            # Tb = [bhat | vb | 1]
            nc.vector.tensor_copy(out=Tb[:, :, D:D + 1], in_=mvb[:, :, 1:2])
            nc.vector.memset(Tb[:, :, D + 1:D + 2], 1.0)

            # ------- rbar = 1/sqrt(mean(va)+mean(vb)+eps) -------------------
            vs2 = work.tile([P, 2], f32, tag="vs2")
            nc.vector.tensor_reduce(
                out=vs2[:, 0:1],
                in_=mva[:, :, 1:2].rearrange("p nb one -> p (nb one)"),
                axis=mybir.AxisListType.X, op=Alu.add)
            nc.vector.tensor_reduce(
                out=vs2[:, 1:2],
                in_=mvb[:, :, 1:2].rearrange("p nb one -> p (nb one)"),
                axis=mybir.AxisListType.X, op=Alu.add)
            vs1 = work.tile([P, 1], f32, tag="vs1")
            nc.vector.tensor_tensor(out=vs1, in0=vs2[:, 0:1], in1=vs2[:, 1:2],
                                    op=Alu.add)
            tpz = scratch[:, 0:P]
            nc.tensor.transpose(tpz[0:1, :], vs1, identity)
            zrow = work.tile([1, P], f32, tag="zrow")
            nc.scalar.activation(out=zrow, in_=tpz[0:1, :], func=Act.Copy)
            zs = work.tile([1, 1], f32, tag="zs")
            nc.vector.tensor_reduce(out=zs, in_=zrow,
                                    axis=mybir.AxisListType.X, op=Alu.add)
            u1 = work.tile([1, 1], f32, tag="u1")
            nc.scalar.activation(out=u1, in_=zs, func=Act.Sqrt,
                                 bias=eps_col[0:1, :], scale=1.0 / T)
            r1 = work.tile([1, 1], f32, tag="r1")
            nc.vector.reciprocal(out=r1, in_=u1)
            nc.tensor.matmul(rbp[:, 0:1], ones1p, r1, start=True, stop=True,
                             skip_group_check=True)
            rbar = work.tile([P, 1], f32, tag="rbar")
            nc.vector.tensor_copy(out=rbar, in_=rbp[:, 0:1])
            if dump == "rbar":
                nc.sync.dma_start(out=dbg_d[:, 0:1], in_=rbar)

            # ------- transposes (PE): a-side, then b-side -------------------
            aT = work.tile([66, NB, P], f32r, tag="aT")
            bT = work.tile([66, NB, P], f32r, tag="bT")
            for blk in range(NB):
                tp = scratch[:, blk * P:blk * P + P]
                nc.tensor.transpose(tp[0:66, :], Ta[:, blk, :], identity)
                nc.scalar.activation(out=aT[:, blk, :],
                                     in_=tp[0:66, :], func=Act.Copy)
            aT_flat = aT.rearrange("k nb p -> k (nb p)")

            # ------- B side heads: EB/bb2 early (gate RB -> B2[deg]) --------
            RB = work.tile([P, NB, deg + 1, D], bf16, tag="RB")
            nc.scalar.activation(out=RB[:, :, deg, :], in_=bhat, func=Act.Exp,
                                 scale=rbar)
            bb2 = work.tile([P, NB, D], bf16, tag="bb2")
            nc.scalar.activation(out=bb2, in_=bhat, func=Act.Square)
            nc.vector.scalar_tensor_tensor(
                out=RB[:, :, deg - 1, :], in0=bhat, scalar=1.0,
                in1=RB[:, :, deg, :], op0=Alu.mult, op1=Alu.mult)
            nc.vector.scalar_tensor_tensor(
                out=RB[:, :, deg - 2, :], in0=bb2, scalar=0.5,
                in1=RB[:, :, deg, :], op0=Alu.mult, op1=Alu.mult)
            if deg >= 3:
                nc.vector.scalar_tensor_tensor(
                    out=RB[:, :, deg - 3, :], in0=bb2, scalar=1.0 / 6,
                    in1=RB[:, :, deg - 1, :], op0=Alu.mult, op1=Alu.mult)
            if deg >= 4:
                nc.vector.scalar_tensor_tensor(
                    out=RB[:, :, deg - 4, :], in0=bb2, scalar=1.0 / 12,
                    in1=RB[:, :, deg - 2, :], op0=Alu.mult, op1=Alu.mult)
            if deg >= 5:
                nc.vector.scalar_tensor_tensor(
                    out=RB[:, :, deg - 5, :], in0=bb2, scalar=1.0 / 20,
                    in1=RB[:, :, deg - 3, :], op0=Alu.mult, op1=Alu.mult)
            # B2[k] = d_k * [B_k .. B_0]; deg and deg-1 built immediately
            # (they gate the first, widest matmuls); the rest interleave
            # with the per-m W chains below.
            B2 = {}

            def build_b2(k):
                B2[k] = work.tile([P, NB, k + 1, D], bf16, tag=f"B2_{k}",
                                  name=f"B2_{k}")
                nc.vector.tensor_scalar(out=B2[k], in0=RB[:, :, deg - k:, :],
                                        scalar1=dk[k], scalar2=None,
                                        op0=Alu.mult)

            build_b2(deg)
            build_b2(deg - 1)
            if dump == "B":
                nc.sync.dma_start(
                    out=dbg_d[:, 0:(deg + 1) * NB * D // 2],
                    in_=RB.rearrange("p nb k d -> p (nb k d)").bitcast(f32))

            # ------- b transposes + var matmuls -> r_m -> W chain ------------
            Dt = [psum.tile([P, 512], f32, tag=f"D{ib}", name=f"D{ib}")
                  for ib in range(NB)]
            rT_all = (work.tile([P, NB, T], f32, tag="rT", name="rT")
                      if dump else None)
            W1s = []
            Wm = []
            for m in range(NB):
                wm = T - P * m
                tp = scratch[:, m * P:m * P + P]
                nc.tensor.transpose(tp[0:66, :], Tb[:, m, :], identity)
                nc.scalar.activation(out=bT[:, m, :],
                                     in_=tp[0:66, :], func=Act.Copy)
                vp = Dt[m][:, 0:T]
                nc.tensor.matmul(vp, bT[:, m, :], aT_flat, start=True,
                                 stop=True, skip_group_check=True)
                ut = rpool.tile([P, T], f32, tag="ut", name=f"u{m}")
                nc.scalar.activation(out=ut, in_=vp, func=Act.Sqrt,
                                     bias=eps_col, scale=1.0)
                rt = rpool.tile([P, T], f32, tag="rt", name=f"r{m}")
                nc.vector.reciprocal(out=rt, in_=ut)
                if dump:
                    nc.vector.tensor_copy(out=rT_all[:, m, :], in_=rt)
                # W_1 = mask*(r - rbar); higher powers by pair products,
                # build order 2,4,3[,5] so the descending consumer unblocks.
                W1 = w1pool.tile([P, T], bf16, tag="W1", name=f"W1_{m}")
                nc.vector.scalar_tensor_tensor(
                    out=W1[:, 0:wm], in0=rt[:, P * m:T], scalar=rbar,
                    in1=mask0[:, 0:wm], op0=Alu.subtract, op1=Alu.mult)
                W1s.append(W1)
                W = {0: mask0, 1: W1}
                build = [(2, 1, 1), (4, 2, 2), (3, 1, 2)]
                if deg == 5:
                    build.append((5, 2, 3))
                for k, a_, b_ in build[:deg - 1]:
                    W[k] = wpool.tile([P, T], bf16, tag="W", name=f"W{k}_{m}")
                    nc.vector.tensor_tensor(
                        out=W[k][:, 0:wm], in0=W[a_][:, 0:wm],
                        in1=W[b_][:, 0:wm], op=Alu.mult)
                Wm.append(W)
                if m < deg - 1:
                    build_b2(deg - 2 - m)

            # ------- A side (consumed by the finals; can lag) ---------------
            A_all = work.tile([P, NB, deg + 1, D], f32, tag="A_all")
            nc.scalar.activation(out=A_all[:, :, 0, :], in_=ahat, func=Act.Exp,
                                 scale=rbar)
            aa2 = work.tile([P, NB, D], f32, tag="aa2")
            nc.scalar.activation(out=aa2, in_=ahat, func=Act.Square)
            nc.vector.scalar_tensor_tensor(
                out=A_all[:, :, 1, :], in0=ahat, scalar=1.0,
                in1=A_all[:, :, 0, :], op0=Alu.mult, op1=Alu.mult)
            nc.vector.scalar_tensor_tensor(
                out=A_all[:, :, 2, :], in0=aa2, scalar=0.5,
                in1=A_all[:, :, 0, :], op0=Alu.mult, op1=Alu.mult)
            if deg >= 3:
                nc.vector.scalar_tensor_tensor(
                    out=A_all[:, :, 3, :], in0=aa2, scalar=1.0 / 6,
                    in1=A_all[:, :, 1, :], op0=Alu.mult, op1=Alu.mult)
            if deg >= 4:
                nc.vector.scalar_tensor_tensor(
                    out=A_all[:, :, 4, :], in0=aa2, scalar=1.0 / 12,
                    in1=A_all[:, :, 2, :], op0=Alu.mult, op1=Alu.mult)
            if deg >= 5:
                nc.vector.scalar_tensor_tensor(
                    out=A_all[:, :, 5, :], in0=aa2, scalar=1.0 / 20,
                    in1=A_all[:, :, 3, :], op0=Alu.mult, op1=Alu.mult)
            if dump == "A":
                nc.sync.dma_start(
                    out=dbg_d[:, 0:(deg + 1) * NB * D],
                    in_=A_all.rearrange("p nb k d -> p (nb k d)"))

            # ------- main accumulation passes (m-major) ---------------------
            Wdump = (work.tile([P, 4, T], f32, tag="Wdump", name="Wdump")
                     if dump == "W" else None)

            def emit_final(m):
                tmp = fin.tile([P, CHUNK], f32, tag="tmp", name=f"tmp{m}")
                nc.vector.tensor_tensor(out=tmp, in0=A_all[:, m, :, :],
                                        in1=Dt[m][:, 0:CHUNK], op=Alu.mult)
                osb = fin.tile([P, D], f32, tag="osb", name=f"osb{m}")
                if m < NB - 1:
                    # off the critical path: binary add tree on idle Pool
                    t3 = fin.tile([P, 2, D], f32, tag="t3", name=f"t3_{m}")
                    nc.gpsimd.tensor_tensor(
                        out=t3, in0=tmp.rearrange("p (s d) -> p s d", s=deg + 1)[:, 0:2, :],
                        in1=tmp.rearrange("p (s d) -> p s d", s=deg + 1)[:, 2:4, :],
                        op=Alu.add)
                    nc.gpsimd.tensor_tensor(out=t3[:, 0, :], in0=t3[:, 0, :],
                                            in1=t3[:, 1, :], op=Alu.add)
                    nc.gpsimd.tensor_tensor(out=osb, in0=t3[:, 0, :],
                                            in1=tmp[:, deg * D:(deg + 1) * D],
                                            op=Alu.add)
                else:
                    nc.vector.tensor_reduce(
                        out=osb,
                        in_=tmp.rearrange("p (s d) -> p d s", s=deg + 1),
                        axis=mybir.AxisListType.X, op=Alu.add)
                nc.sync.dma_start(out=out_d[m * P:(m + 1) * P, :], in_=osb)

            for m in range(NB):
                W = Wm[m]
                # pass 0 runs k descending: k=deg covers the full CHUNK with
                # start=True, then strictly-nested prefix accumulates.
                korder = (range(deg, -1, -1) if m == 0 else range(deg + 1))
                for k in korder:
                    for ib in range(m, NB):
                        lhsT = W[k][:, (ib - m) * P:(ib - m) * P + P]
                        start = (m == 0 and k == deg)
                        last = (m == ib and k == (deg if m else 0))
                        nc.tensor.matmul(Dt[ib][:, 0:(k + 1) * D], lhsT,
                                         B2[k][:, m, :, :], start=start,
                                         stop=last, skip_group_check=True)
                if dump == "W" and m == 0:
                    for k in range(1, min(deg + 1, 5)):
                        nc.vector.tensor_copy(out=Wdump[:, k - 1, :],
                                              in_=W[k][:, 0:T])
                    nc.sync.dma_start(out=dbg_d[:], in_=Wdump.rearrange(
                        "p f t -> p (f t)"))
                emit_final(m)

            if dump == "r":
                nc.sync.dma_start(out=dbg_d[:], in_=rT_all.rearrange(
                    "p nb t -> p (nb t)"))
            if dump == "D":
                for ib in range(2):
                    dcp = fin.tile([P, CHUNK], f32, tag="dcp", name=f"dcp{ib}")
                    nc.vector.tensor_copy(out=dcp, in_=Dt[ib][:, 0:CHUNK])
                    nc.sync.dma_start(out=dbg_d[:, ib * CHUNK:(ib + 1) * CHUNK],
                                      in_=dcp)

    _split_multi_waits(nc, mybir)
    return nc


def _split_multi_waits(nc, mybir):
    """TRN2 TPB instructions have a single sync-wait slot; walrus cannot
    split >1 wait for several structs. Use the bacc rust pass to split
    them into EventSemaphore instructions."""
    import bass_rust as _bass_rust
    _bass_rust.generate_event_semaphores(nc)
    # walrus rejects wait-only EventSemaphore encodings ("ISA wrong length")
    # and requires update_value == 1. Give each wait-carrier a +1 update of a
    # scratch semaphore nothing ever waits on.
    used = set()
    for f in nc.m.functions:
        for blk in f.blocks:
            for inst in blk.instructions:
                si = getattr(inst, "sync_info", None)
                if si is not None:
                    for w in (si.on_wait or []):
                        used.add(w.id)
                    for u in (si.on_update or []):
                        used.add(u.id)
    scratch = next(s for s in nc._kernel_sem_range if s not in used)
    for f in nc.m.functions:
        for blk in f.blocks:
            for inst in blk.instructions:
                if isinstance(inst, mybir.InstEventSemaphore):
                    si = inst.sync_info
                    if si is not None and si.on_wait and not si.on_update:
                        si.on_update = [_bass_rust.SyncUpdate(
                            sync_type='semaphore', id=scratch,
                            ant_name='wsplit_scratch',
                            update_mode='sem-inc', update_value=1,
                            update_reg=None)]
    # Drop end-of-kernel EVENT_SEMAPHORE_RANGE_CLEAR (opcode 0xb0): this
    # walrus build rejects its encoding ("ISA wrong length"), and the kernel
    # preamble re-clears all kernel semaphores on every run anyway.
    for f in nc.m.functions:
        for blk in f.blocks:
            blk.instructions[:] = [
                inst for inst in blk.instructions
                if not (isinstance(inst, mybir.InstISA)
                        and getattr(inst, "isa_opcode", None) == 0xb0
                        and not (inst.sync_info and
                                 (inst.sync_info.on_wait or
                                  inst.sync_info.on_update)))
            ]


def _get_nc(deg=DEG, dump=None):
    key = ("nc", deg, dump)
    if key not in _cached:
        _cached[key] = _build_nc(deg, dump)
    return _cached[key]


def kernel(a, b, num_head=8, head_size=64, **kwargs):
    from concourse.bass_utils import run_bass_kernel_spmd

    a = np.asarray(a)
    b = np.asarray(b)
    nc = _get_nc()
    in_maps = []
    for h in range(H):
        in_maps.append({
            "ah": np.ascontiguousarray(a[0, :, h * D:(h + 1) * D], dtype=np.float32),
            "bh": np.ascontiguousarray(b[0, :, h * D:(h + 1) * D], dtype=np.float32),
        })
    res = run_bass_kernel_spmd(nc, in_maps, list(range(H)))
    full = np.concatenate([res.results[h]["out"] for h in range(H)], axis=-1)
    return full[None].astype(np.float32)


if __name__ == "__main__":
    sys.path.insert(0, "/opt/trn_rl_repo")
    _build_nc()
    print("build OK")


# revision 19
# speedup vs baseline: 2.0772x; 1.0667x over previous
"""Trainium2 Bass kernel for nn_BAR_86045374808446 (sparse_attention).

Math per head h (one head per NeuronCore, 8 cores):
  s[i,j,d] = ahat_i[d] + bhat_j[d]          (ahat/bhat are d-mean-centered)
  var[i,j] = va[i] + vb[j] + (2/D)<ahat_i, bhat_j>     (one PE matmul per block)
  r[i,j]   = 1/sqrt(var + eps)
  out[i,d] = sum_{j<=i} exp(s[i,j,d] * r[i,j])

Factorization (polynomial P(x) ~ exp(x) on the observed x-range):
  exp(s*r) = exp(ahat*rbar) * exp(bhat*rbar) * exp(s*w),  w = r - rbar
  exp(s*w) ~ P(s*w) = sum_k c_k (s*w)^k
  (s*w)^k  = sum_{p+e=k} k!/(p!e!) ahat^p bhat^e w^k
  => out = sum_p A_p (*) sum_k (M*w^k)^T @ (d_k * B_{k-p}),  d_k = c_k k!
  with A_p = ahat^p/p! * exp(ahat*rbar)  [i,d],
       B_e = bhat^e/e! * exp(bhat*rbar)  [j,d],
  so the T^2*D work is bf16 PSUM-accumulated matmuls on the TensorEngine,
  and the polynomial coefficients ride on pre-scaled bf16 rhs copies (B2).
  rbar = 1/sqrt(mean va + mean vb + eps) -- picked to center the x-range;
  c_k are a Chebyshev fit of exp on that range (error budget 2e-2 rel).
"""

import math
import sys

import numpy as np

for _p in ("/opt/trn_rl_repo", "/root/.axon_site/_ro/trn_rl_repo"):
    if _p not in sys.path:
        sys.path.insert(0, _p)

T, D, H, P, NB = 512, 64, 8, 128, 4
EPS = 1e-5
DEG = 4
COEF = {
    4: [0.99963261, 0.99058825, 0.50079216, 0.18677153, 0.043321831],
    5: [1.00029, 0.99982237, 0.49719599, 0.16689019, 0.045660714,
        0.0085691588],
}

_cached = {}


def _build_nc(deg=DEG, dump=None):
    import concourse.bass as bass
    import concourse.mybir as mybir
    from concourse.tile import TileContext
    from concourse.masks import make_identity

    f32 = mybir.dt.float32
    f32r = mybir.dt.float32r
    bf16 = mybir.dt.bfloat16
    Alu = mybir.AluOpType
    Act = mybir.ActivationFunctionType

    coef = COEF[deg]
    dk = [float(coef[k]) * math.factorial(k) for k in range(deg + 1)]
    CHUNK = (deg + 1) * D

    nc = bass.Bass()
    ah_d = nc.declare_dram_parameter("ah", [T, D], f32, isOutput=False)
    bh_d = nc.declare_dram_parameter("bh", [T, D], f32, isOutput=False)
    out_d = nc.declare_dram_parameter("out", [T, D], f32, isOutput=True)
    dbg_d = (nc.declare_dram_parameter("dbg", [P, 4 * T], f32, isOutput=True)
             if dump else None)

    with TileContext(nc) as tc:
        with (
            tc.tile_pool(name="const", bufs=1) as constp,
            tc.tile_pool(name="work", bufs=1) as work,
            tc.tile_pool(name="wpool", bufs=8) as wpool,
            tc.tile_pool(name="w1pool", bufs=4) as w1pool,
            tc.tile_pool(name="rpool", bufs=2) as rpool,
            tc.tile_pool(name="fin", bufs=4) as fin,
            tc.tile_pool(name="psum", bufs=1, space="PSUM") as psum,
            tc.tile_pool(name="psumR", bufs=1, space="PSUM") as psumR,
        ):
            # ------- loads: two independent DMA queues (SP + Pool) ----------
            Asb = work.tile([P, NB, D], f32, tag="Asb")
            Bsb = work.tile([P, NB, D], f32, tag="Bsb")
            nc.sync.dma_start(out=Asb,
                              in_=ah_d[:].rearrange("(nb p) d -> p nb d", p=P))
            nc.scalar.dma_start(out=Bsb,
                                in_=bh_d[:].rearrange("(nb p) d -> p nb d", p=P))

            # ------- constants ----------------------------------------------
            identity = constp.tile([P, P], f32, tag="ident")
            make_identity(nc, identity)
            eps_col = constp.tile([P, 1], f32, tag="eps")
            nc.vector.memset(eps_col, EPS)
            ones1p = constp.tile([1, P], f32, tag="ones1p")
            nc.vector.memset(ones1p, 1.0)
            ones_col = constp.tile([P, 1], f32, tag="ones_col")
            nc.vector.memset(ones_col, 1.0)
            ones_bf = constp.tile([P, T], bf16, tag="ones_bf")
            nc.gpsimd.memset(ones_bf, 1.0)
            # causal mask (j<=i within-block pattern; same for every m)
            mask0 = constp.tile([P, T], bf16, tag="mask0")
            nc.gpsimd.affine_select(
                out=mask0, in_=ones_bf, compare_op=Alu.is_ge, fill=0.0,
                base=0, channel_multiplier=-1, pattern=[[1, T]])
            # warm ACT tables off the critical path
            warm = constp.tile([P, 1], f32, tag="warm")
            nc.scalar.activation(out=warm, in_=eps_col, func=Act.Sqrt)
            nc.scalar.activation(out=warm, in_=eps_col, func=Act.Exp)
            nc.scalar.activation(out=warm, in_=eps_col, func=Act.Square)
            nc.scalar.activation(out=warm, in_=eps_col, func=Act.Identity)

            # PSUM scratch banks for transposes; rbar ops in their own bank
            scratch = psum.tile([P, 512], f32, tag="scratch")
            scratch2 = psum.tile([P, 512], f32, tag="scratch2")
            rbp = psumR.tile([P, 8], f32, tag="rbp")

            # PE pstate warm-up: back-to-back identity transposes keep the
            # tensor engine ramping while the DMA/stats preamble runs.
            for i in range(10):
                tp = scratch[:, 256 + (i % 2) * P:256 + (i % 2) * P + P]
                nc.tensor.transpose(tp, identity, identity)

            # ------- stats + centering (a first, then b) --------------------
            mva = work.tile([P, NB, 2], f32, tag="mva")
            mvb = work.tile([P, NB, 2], f32, tag="mvb")
            negmua = work.tile([P, NB, 1], f32, tag="negmua")
            negmua2 = work.tile([P, NB, 1], f32, tag="negmua2")
            negmub = work.tile([P, NB, 1], f32, tag="negmub")
            ahat = work.tile([P, NB, D], f32, tag="ahat")
            Ta = work.tile([P, NB, 66], f32, tag="Ta")
            Tb = work.tile([P, NB, 66], f32, tag="Tb")
            bhat = Tb[:, :, 0:D]

            for blk in range(NB):
                sa = work.tile([P, 6], f32, tag="bnsA", name=f"bnsA{blk}")
                nc.vector.bn_stats(out=sa, in_=Asb[:, blk, :])
                nc.vector.bn_aggr(out=mva[:, blk, :], in_=sa)
            nc.vector.tensor_scalar(out=negmua, in0=mva[:, :, 0:1], scalar1=-1.0,
                                    scalar2=None, op0=Alu.mult)
            nc.vector.tensor_scalar(out=negmua2, in0=mva[:, :, 0:1],
                                    scalar1=-2.0 / D, scalar2=None,
                                    op0=Alu.mult)
            # Ta = [(2/D)*ahat | 1 | va] -- centered+scaled directly by ACT
            for blk in range(NB):
                nc.scalar.activation(out=Ta[:, blk, 0:D], in_=Asb[:, blk, :],
                                     func=Act.Identity, bias=negmua2[:, blk, :],
                                     scale=2.0 / D)
            nc.vector.memset(Ta[:, :, D:D + 1], 1.0)
            nc.vector.tensor_copy(out=Ta[:, :, D + 1:D + 2], in_=mva[:, :, 1:2])
            for blk in range(NB):
                sb = work.tile([P, 6], f32, tag="bnsB", name=f"bnsB{blk}")
                nc.vector.bn_stats(out=sb, in_=Bsb[:, blk, :])
                nc.vector.bn_aggr(out=mvb[:, blk, :], in_=sb)
            nc.vector.tensor_scalar(out=negmub, in0=mvb[:, :, 0:1], scalar1=-1.0,
                                    scalar2=None, op0=Alu.mult)
            for blk in range(NB):
                nc.scalar.activation(out=Tb[:, blk, 0:D], in_=Bsb[:, blk, :],
                                     func=Act.Identity, bias=negmub[:, blk, :])
            # Tb = [bhat | vb | 1]
            nc.vector.tensor_copy(out=Tb[:, :, D:D + 1], in_=mvb[:, :, 1:2])
            nc.vector.memset(Tb[:, :, D + 1:D + 2], 1.0)

            # ------- rbar = 1/sqrt(mean(va)+mean(vb)+eps) -------------------
            vs2 = work.tile([P, 2], f32, tag="vs2")
            nc.vector.tensor_reduce(
                out=vs2[:, 0:1],
                in_=mva[:, :, 1:2].rearrange("p nb one -> p (nb one)"),
                axis=mybir.AxisListType.X, op=Alu.add)
            nc.vector.tensor_reduce(
                out=vs2[:, 1:2],
                in_=mvb[:, :, 1:2].rearrange("p nb one -> p (nb one)"),
                axis=mybir.AxisListType.X, op=Alu.add)
            vs1 = work.tile([P, 1], f32, tag="vs1")
            nc.vector.tensor_tensor(out=vs1, in0=vs2[:, 0:1], in1=vs2[:, 1:2],
                                    op=Alu.add)
            # cross-partition sum: [P,1]^T @ ones -> [1,1] in one matmul
            nc.tensor.matmul(rbp[0:1, 1:2], vs1, ones_col, start=True,
                             stop=True, skip_group_check=True)
            u1 = work.tile([1, 1], f32, tag="u1")
            nc.scalar.activation(out=u1, in_=rbp[0:1, 1:2], func=Act.Sqrt,
                                 bias=eps_col[0:1, :], scale=1.0 / T)
            r1 = work.tile([1, 1], f32, tag="r1")
            nc.vector.reciprocal(out=r1, in_=u1)
            nc.tensor.matmul(rbp[:, 0:1], ones1p, r1, start=True, stop=True,
                             skip_group_check=True)
            rbar = work.tile([P, 1], f32, tag="rbar")
            nc.vector.tensor_copy(out=rbar, in_=rbp[:, 0:1])
            if dump == "rbar":
                nc.sync.dma_start(out=dbg_d[:, 0:1], in_=rbar)

            # ------- transposes (PE): a-side, then b-side -------------------
            aT = work.tile([66, NB, P], f32r, tag="aT")
            bT = work.tile([66, NB, P], f32r, tag="bT")
            for blk in range(NB):
                tp = scratch[:, blk * P:blk * P + P]
                nc.tensor.transpose(tp[0:66, :], Ta[:, blk, :], identity)
                nc.scalar.activation(out=aT[:, blk, :],
                                     in_=tp[0:66, :], func=Act.Copy)
            aT_flat = aT.rearrange("k nb p -> k (nb p)")

            # ------- B side heads: EB + prescaled squares (gate RB/B2) ------
            # sq2b = bhat^2/2 (Square of bhat*sqrt(1/2)); sq6b = bhat^2/12.
            # Odd chain (stt) on DVE; even chain (tensor_tensor) on Pool.
            RB = work.tile([P, NB, deg + 1, D], bf16, tag="RB")
            nc.scalar.activation(out=RB[:, :, deg, :], in_=bhat, func=Act.Exp,
                                 scale=rbar)
            sq2b = work.tile([P, NB, D], bf16, tag="sq2b")
            nc.scalar.activation(out=sq2b, in_=bhat, func=Act.Square,
                                 scale=math.sqrt(0.5))
            sq6b = work.tile([P, NB, D], bf16, tag="sq6b")
            nc.scalar.activation(out=sq6b, in_=bhat, func=Act.Square,
                                 scale=math.sqrt(1.0 / 12))
            nc.vector.scalar_tensor_tensor(
                out=RB[:, :, deg - 1, :], in0=bhat, scalar=1.0,
                in1=RB[:, :, deg, :], op0=Alu.mult, op1=Alu.mult)
            nc.gpsimd.tensor_tensor(out=RB[:, :, deg - 2, :], in0=sq2b,
                                    in1=RB[:, :, deg, :], op=Alu.mult)
            if deg >= 3:
                nc.vector.scalar_tensor_tensor(
                    out=RB[:, :, deg - 3, :], in0=sq2b, scalar=1.0 / 3,
                    in1=RB[:, :, deg - 1, :], op0=Alu.mult, op1=Alu.mult)
            if deg >= 4:
                nc.gpsimd.tensor_tensor(out=RB[:, :, deg - 4, :], in0=sq6b,
                                        in1=RB[:, :, deg - 2, :], op=Alu.mult)
            if deg >= 5:
                nc.vector.scalar_tensor_tensor(
                    out=RB[:, :, deg - 5, :], in0=sq2b, scalar=1.0 / 10,
                    in1=RB[:, :, deg - 3, :], op0=Alu.mult, op1=Alu.mult)
            # B2[k] = d_k * [B_k .. B_0]; deg and deg-1 built immediately
            # (they gate the first, widest matmuls); the rest interleave
            # with the per-m W chains below.
            B2 = {}

            def build_b2(k):
                B2[k] = work.tile([P, NB, k + 1, D], bf16, tag=f"B2_{k}",
                                  name=f"B2_{k}")
                nc.vector.tensor_scalar(out=B2[k], in0=RB[:, :, deg - k:, :],
                                        scalar1=dk[k], scalar2=None,
                                        op0=Alu.mult)

            build_b2(deg)
            build_b2(deg - 1)
            if dump == "B":
                nc.sync.dma_start(
                    out=dbg_d[:, 0:(deg + 1) * NB * D // 2],
                    in_=RB.rearrange("p nb k d -> p (nb k d)").bitcast(f32))

            # ------- b transposes + var matmuls -> r_m -> W chain ------------
            Dt = [psum.tile([P, 512], f32, tag=f"D{ib}", name=f"D{ib}")
                  for ib in range(NB)]
            rT_all = (work.tile([P, NB, T], f32, tag="rT", name="rT")
                      if dump else None)
            W1s = []
            Wm = []
            for m in range(NB):
                wm = T - P * m
                tp = scratch2[:, m * P:m * P + P]
                nc.tensor.transpose(tp[0:66, :], Tb[:, m, :], identity)
                nc.scalar.activation(out=bT[:, m, :],
                                     in_=tp[0:66, :], func=Act.Copy)
                vp = Dt[m][:, 0:T]
                nc.tensor.matmul(vp, bT[:, m, :], aT_flat, start=True,
                                 stop=True, skip_group_check=True)
                ut = rpool.tile([P, T], f32, tag="ut", name=f"u{m}")
                nc.scalar.activation(out=ut, in_=vp, func=Act.Sqrt,
                                     bias=eps_col, scale=1.0)
                rt = rpool.tile([P, T], f32, tag="rt", name=f"r{m}")
                nc.vector.reciprocal(out=rt, in_=ut)
                if dump:
                    nc.vector.tensor_copy(out=rT_all[:, m, :], in_=rt)
                # W_1 = mask*(r - rbar); higher powers by pair products,
                # build order 2,4,3[,5] so the descending consumer unblocks.
                W1 = w1pool.tile([P, T], bf16, tag="W1", name=f"W1_{m}")
                nc.vector.scalar_tensor_tensor(
                    out=W1[:, 0:wm], in0=rt[:, P * m:T], scalar=rbar,
                    in1=mask0[:, 0:wm], op0=Alu.subtract, op1=Alu.mult)
                W1s.append(W1)
                W = {0: mask0, 1: W1}
                build = [(2, 1, 1), (4, 2, 2), (3, 1, 2)]
                if deg == 5:
                    build.append((5, 2, 3))
                for k, a_, b_ in build[:deg - 1]:
                    W[k] = wpool.tile([P, T], bf16, tag="W", name=f"W{k}_{m}")
                    nc.vector.tensor_tensor(
                        out=W[k][:, 0:wm], in0=W[a_][:, 0:wm],
                        in1=W[b_][:, 0:wm], op=Alu.mult)
                Wm.append(W)
                if m < deg - 1:
                    build_b2(deg - 2 - m)

            # ------- A side (consumed by the finals; can lag) ---------------
            # ahat (plain centered a) is only needed from here on.
            for blk in range(NB):
                nc.scalar.activation(out=ahat[:, blk, :], in_=Asb[:, blk, :],
                                     func=Act.Identity, bias=negmua[:, blk, :])
            A_all = work.tile([P, NB, deg + 1, D], f32, tag="A_all")
            nc.scalar.activation(out=A_all[:, :, 0, :], in_=ahat, func=Act.Exp,
                                 scale=rbar)
            sq2a = work.tile([P, NB, D], f32, tag="sq2a")
            nc.scalar.activation(out=sq2a, in_=ahat, func=Act.Square,
                                 scale=math.sqrt(0.5))
            sq6a = work.tile([P, NB, D], f32, tag="sq6a")
            nc.scalar.activation(out=sq6a, in_=ahat, func=Act.Square,
                                 scale=math.sqrt(1.0 / 12))
            nc.vector.scalar_tensor_tensor(
                out=A_all[:, :, 1, :], in0=ahat, scalar=1.0,
                in1=A_all[:, :, 0, :], op0=Alu.mult, op1=Alu.mult)
            nc.gpsimd.tensor_tensor(out=A_all[:, :, 2, :], in0=sq2a,
                                    in1=A_all[:, :, 0, :], op=Alu.mult)
            if deg >= 3:
                nc.vector.scalar_tensor_tensor(
                    out=A_all[:, :, 3, :], in0=sq2a, scalar=1.0 / 3,
                    in1=A_all[:, :, 1, :], op0=Alu.mult, op1=Alu.mult)
            if deg >= 4:
                nc.gpsimd.tensor_tensor(out=A_all[:, :, 4, :], in0=sq6a,
                                        in1=A_all[:, :, 2, :], op=Alu.mult)
            if deg >= 5:
                nc.vector.scalar_tensor_tensor(
                    out=A_all[:, :, 5, :], in0=sq2a, scalar=1.0 / 10,
                    in1=A_all[:, :, 3, :], op0=Alu.mult, op1=Alu.mult)
            if dump == "A":
                nc.sync.dma_start(
                    out=dbg_d[:, 0:(deg + 1) * NB * D],
                    in_=A_all.rearrange("p nb k d -> p (nb k d)"))

            # ------- main accumulation passes (m-major) ---------------------
            Wdump = (work.tile([P, 4, T], f32, tag="Wdump", name="Wdump")
                     if dump == "W" else None)

            def emit_final(m):
                tmp = fin.tile([P, CHUNK], f32, tag="tmp", name=f"tmp{m}")
                nc.vector.tensor_tensor(out=tmp, in0=A_all[:, m, :, :],
                                        in1=Dt[m][:, 0:CHUNK], op=Alu.mult)
                osb = fin.tile([P, D], f32, tag="osb", name=f"osb{m}")
                if m < NB - 1:
                    # off the critical path: binary add tree on idle Pool
                    t3 = fin.tile([P, 2, D], f32, tag="t3", name=f"t3_{m}")
                    nc.gpsimd.tensor_tensor(
                        out=t3, in0=tmp.rearrange("p (s d) -> p s d", s=deg + 1)[:, 0:2, :],
                        in1=tmp.rearrange("p (s d) -> p s d", s=deg + 1)[:, 2:4, :],
                        op=Alu.add)
                    nc.gpsimd.tensor_tensor(out=t3[:, 0, :], in0=t3[:, 0, :],
                                            in1=t3[:, 1, :], op=Alu.add)
                    nc.gpsimd.tensor_tensor(out=osb, in0=t3[:, 0, :],
                                            in1=tmp[:, deg * D:(deg + 1) * D],
                                            op=Alu.add)
                else:
                    nc.vector.tensor_reduce(
                        out=osb,
                        in_=tmp.rearrange("p (s d) -> p d s", s=deg + 1),
                        axis=mybir.AxisListType.X, op=Alu.add)
                eng = nc.sync if m % 2 == 0 else nc.scalar
                eng.dma_start(out=out_d[m * P:(m + 1) * P, :], in_=osb)

            for m in range(NB):
                W = Wm[m]
                # pass 0 runs k descending: k=deg covers the full CHUNK with
                # start=True, then strictly-nested prefix accumulates.
                korder = (range(deg, -1, -1) if m == 0 else range(deg + 1))
                for k in korder:
                    for ib in range(m, NB):
                        lhsT = W[k][:, (ib - m) * P:(ib - m) * P + P]
                        start = (m == 0 and k == deg)
                        last = (m == ib and k == (deg if m else 0))
                        nc.tensor.matmul(Dt[ib][:, 0:(k + 1) * D], lhsT,
                                         B2[k][:, m, :, :], start=start,
                                         stop=last, skip_group_check=True)
                if dump == "W" and m == 0:
                    for k in range(1, min(deg + 1, 5)):
                        nc.vector.tensor_copy(out=Wdump[:, k - 1, :],
                                              in_=W[k][:, 0:T])
                    nc.sync.dma_start(out=dbg_d[:], in_=Wdump.rearrange(
                        "p f t -> p (f t)"))
                emit_final(m)

            if dump == "r":
                nc.sync.dma_start(out=dbg_d[:], in_=rT_all.rearrange(
                    "p nb t -> p (nb t)"))
            if dump == "D":
                for ib in range(2):
                    dcp = fin.tile([P, CHUNK], f32, tag="dcp", name=f"dcp{ib}")
                    nc.vector.tensor_copy(out=dcp, in_=Dt[ib][:, 0:CHUNK])
                    nc.sync.dma_start(out=dbg_d[:, ib * CHUNK:(ib + 1) * CHUNK],
                                      in_=dcp)

    _split_multi_waits(nc, mybir)
    return nc


def _split_multi_waits(nc, mybir):
    """TRN2 TPB instructions have a single sync-wait slot; walrus cannot
    split >1 wait for several structs. Use the bacc rust pass to split
    them into EventSemaphore instructions."""
    import bass_rust as _bass_rust
    _bass_rust.generate_event_semaphores(nc)
    # walrus rejects wait-only EventSemaphore encodings ("ISA wrong length")
    # and requires update_value == 1. Give each wait-carrier a +1 update of a
    # scratch semaphore nothing ever waits on.
    used = set()
    for f in nc.m.functions:
        for blk in f.blocks:
            for inst in blk.instructions:
                si = getattr(inst, "sync_info", None)
                if si is not None:
                    for w in (si.on_wait or []):
                        used.add(w.id)
                    for u in (si.on_update or []):
                        used.add(u.id)
    scratch = next(s for s in nc._kernel_sem_range if s not in used)
    for f in nc.m.functions:
        for blk in f.blocks:
            for inst in blk.instructions:
                if isinstance(inst, mybir.InstEventSemaphore):
                    si = inst.sync_info
                    if si is not None and si.on_wait and not si.on_update:
                        si.on_update = [_bass_rust.SyncUpdate(
                            sync_type='semaphore', id=scratch,
                            ant_name='wsplit_scratch',
                            update_mode='sem-inc', update_value=1,
                            update_reg=None)]
    # Drop end-of-kernel EVENT_SEMAPHORE_RANGE_CLEAR (opcode 0xb0): this
    # walrus build rejects its encoding ("ISA wrong length"), and the kernel
    # preamble re-clears all kernel semaphores on every run anyway.
    for f in nc.m.functions:
        for blk in f.blocks:
            blk.instructions[:] = [
                inst for inst in blk.instructions
                if not (isinstance(inst, mybir.InstISA)
                        and getattr(inst, "isa_opcode", None) == 0xb0
                        and not (inst.sync_info and
                                 (inst.sync_info.on_wait or
                                  inst.sync_info.on_update)))
            ]


def _get_nc(deg=DEG, dump=None):
    key = ("nc", deg, dump)
    if key not in _cached:
        _cached[key] = _build_nc(deg, dump)
    return _cached[key]


def kernel(a, b, num_head=8, head_size=64, **kwargs):
    from concourse.bass_utils import run_bass_kernel_spmd

    a = np.asarray(a)
    b = np.asarray(b)
    nc = _get_nc()
    in_maps = []
    for h in range(H):
        in_maps.append({
            "ah": np.ascontiguousarray(a[0, :, h * D:(h + 1) * D], dtype=np.float32),
            "bh": np.ascontiguousarray(b[0, :, h * D:(h + 1) * D], dtype=np.float32),
        })
    res = run_bass_kernel_spmd(nc, in_maps, list(range(H)))
    full = np.concatenate([res.results[h]["out"] for h in range(H)], axis=-1)
    return full[None].astype(np.float32)


if __name__ == "__main__":
    sys.path.insert(0, "/opt/trn_rl_repo")
    _build_nc()
    print("build OK")
